# revision 34
# baseline (speedup 1.0000x reference)
"""GAT (2-layer, PyG-style) Trainium2 Bass kernel, 8-core SPMD. v4.

Strategy (dst-per-partition, reduction-based aggregation):
- Host renumbers nodes: sort by (degree desc, lo-count), deal rank-blocks
  of 128 to (core, position) so all 8 cores' blocks at the same position
  have near-equal max degree.  Core c owns contiguous new-ids
  [c*BPC*128, (c+1)*BPC*128).  Partition p of block b IS dst node; its
  edges lie along the free dim as [lo-src | pad | hi-src | pad], padded
  to per-position (Glo, Ghi) maxima over cores (the compiled program is
  shared by all cores).
- a_dst is a per-partition scalar (block-local SBUF table from phase 0)
  => no a_dst gather, no one-hot scatter matmuls.  Segment softmax +
  scatter-add become per-partition row ops + a free-dim tree reduction.
- h/a_src rows are fetched with InstDMAGatherAnt (int16 idx) from node
  tables split at row 25088 so both halves' indices fit int16. Pad slots
  gather a sentinel row whose a_src is -100 => exp()==0 in fp16, so
  padding contributes exactly zero to message and denominator sums.
- Logits are exp-shifted by -2 (cancels in softmax; keeps fp16 finite).
  ELU's "-1" is dropped (eluplus = relu(x)+exp(min(x,0))) and corrected
  at the end: out -= colsum(W2); the induced constant layer-2 logit
  shift c0 is subtracted pre-leaky_relu.  Sentinel layer-2 a_src is
  forced to -1000 via a padmask input baked in before the AllGather.
- One AllGather of the 128-col fp16 layer-2 node table is the only
  collective.
"""

import numpy as np

import concourse.bacc as bacc
import concourse.mybir as mybir
import concourse.tile as tile
from concourse.bass_utils import run_bass_kernel_spmd
from bass_rust import add_dep_helper


def _dep(a, b, reason):
    ia = a.ins if hasattr(a, "ins") else a
    ib = b.ins if hasattr(b, "ins") else b
    add_dep_helper(ia, ib, reason=reason)


P = 128
NCORES = 8
EPS = 1e-16
NEG_SLOPE = 0.2
SPLIT = 25088               # node-table split so gather idx fits int16
F32 = mybir.dt.float32
F16 = mybir.dt.float16
I32 = mybir.dt.int32
I16 = mybir.dt.int16
AF = mybir.ActivationFunctionType
ALU = mybir.AluOpType

MAXCOLS = 8                 # <=8 cols (1024 idx) per gather call
GCAP = 24                   # max grid columns per processing segment


class Cfg:
    def __init__(self, n_nodes, glo, ghi, c_in=128, h1=8, ch1=32, c2=64,
                 ncores=NCORES):
        self.n = n_nodes
        self.c_in = c_in
        self.h1 = h1
        self.ch1 = ch1
        self.hc1 = h1 * ch1          # 256
        self.c2 = c2
        self.ncores = ncores
        self.bpc = -(-n_nodes // (P * ncores))      # 49
        self.npad = ncores * self.bpc * P
        self.nblk = ncores * self.bpc
        self.t1w = 384               # [h 256 | asrc 8 | adst 8 | junk]
        self.t2w = 128               # [h2 64 | asrc2 1 | adst2 1 | junk]
        self.glo = list(glo)         # per block position, len bpc
        self.ghi = list(ghi)
        self.g = [a + b for a, b in zip(self.glo, self.ghi)]
        self.sg = sum(self.g)
        self.gmax = max(self.g)


_GQ = [0]


def build_program(cfg):
    nc = bacc.Bacc(None, num_devices=cfg.ncores, num_swdge_queues=4,
                   dynamic_dma_scratch_size=16384)
    HC1, H1, CH1, C2 = cfg.hc1, cfg.h1, cfg.ch1, cfg.c2
    T1W, T2W, BPC = cfg.t1w, cfg.t2w, cfg.bpc
    NBLK, NPAD = cfg.nblk, cfg.npad
    G1 = 8
    G0 = -(-BPC // G1)              # phase-0 groups of 8 blocks

    # ---- I/O ----
    xt = nc.dram_tensor("xt", [cfg.c_in, NPAD], F16, kind="ExternalInput")
    w1aug = nc.dram_tensor("w1aug", [cfg.c_in, 272], F16, kind="ExternalInput")
    w2aug = nc.dram_tensor("w2aug", [HC1, 66], F16, kind="ExternalInput")
    b1b = nc.dram_tensor("b1b", [P, HC1], F16, kind="ExternalInput")
    b2b = nc.dram_tensor("b2b", [P, C2], F32, kind="ExternalInput")
    shifts = nc.dram_tensor("shifts", [P, 2], F32, kind="ExternalInput")
    ident = nc.dram_tensor("ident", [P, P], F16, kind="ExternalInput")
    padmask = nc.dram_tensor("padmask", [P, BPC], F16, kind="ExternalInput")
    eidx = nc.dram_tensor("eidx", [P, 8 * cfg.sg], I16, kind="ExternalInput")
    aidx = nc.dram_tensor("aidx", [P, 2 * BPC * 8], I16, kind="ExternalInput")
    sentadst = nc.dram_tensor("sentadst", [P, H1], F16, kind="ExternalInput")
    out = nc.dram_tensor("out", [BPC * P, C2], F32, kind="ExternalOutput")

    # ---- internal DRAM ----
    t1 = nc.dram_tensor("t1", [NPAD, T1W], F16)
    cc_in = nc.dram_tensor("cc_in", [BPC * P, T2W], F16)
    t2 = nc.dram_tensor("t2", [NPAD, T2W], F16, addr_space="Shared")

    groups = [list(range(cfg.ncores))]
    GM = cfg.gmax

    with tile.TileContext(nc) as tc:
        with (
            tc.tile_pool(name="const", bufs=1) as cpool,
            tc.tile_pool(name="p1", bufs=2) as p1pool,
            tc.tile_pool(name="gat", bufs=4) as gpool,
            tc.tile_pool(name="gat2", bufs=5) as g2pool,
            tc.tile_pool(name="blk", bufs=3) as bpool,
            tc.tile_pool(name="fin", bufs=4) as opool,
            tc.tile_pool(name="ps", bufs=2, space="PSUM") as ps,
        ):
            # ---------------- constants ----------------
            ident_s = cpool.tile([P, P], F16)
            nc.sync.dma_start(out=ident_s[:], in_=ident[:])
            w1aug_s = cpool.tile([P, 272], F16)
            nc.sync.dma_start(out=w1aug_s[:], in_=w1aug[:])
            w2aug_s = []
            for j in range(HC1 // P):
                wg = cpool.tile([P, 66], F16, tag=f"w2aug{j}")
                nc.sync.dma_start(out=wg[:], in_=w2aug[j * P:(j + 1) * P, :])
                w2aug_s.append(wg)
            b1b_s = cpool.tile([P, HC1], F16)
            nc.sync.dma_start(out=b1b_s[:], in_=b1b[:])
            b2b_s = cpool.tile([P, C2], F32)
            nc.sync.dma_start(out=b2b_s[:], in_=b2b[:])
            shifts_s = cpool.tile([P, 2], F32)
            nc.sync.dma_start(out=shifts_s[:], in_=shifts[:])
            padmask_s = cpool.tile([P, BPC], F16)
            nc.sync.dma_start(out=padmask_s[:], in_=padmask[:])
            aidx_s = cpool.tile([P, 2 * BPC * 8], I16)
            nc.sync.dma_start(out=aidx_s[:], in_=aidx[:])
            sentadst_s = cpool.tile([P, H1], F16)
            nc.sync.dma_start(out=sentadst_s[:], in_=sentadst[:])
            # persistent per-core tables
            A1 = cpool.tile([P, BPC * H1], F16)       # own-node a_dst
            A2 = cpool.tile([P, BPC], F16)            # own-node a_dst2

            # ------------- phase 1: full node table (replicated) ---------
            t1_writes = []
            for grp in range(NBLK // G1):
                B0 = grp * G1
                xg = p1pool.tile([P, G1 * P], F16, tag="xg")
                nc.sync.dma_start(out=xg[:], in_=xt[:, B0 * P:(B0 + G1) * P])
                rows = p1pool.tile([P, G1 * 272], F16, tag="rows")
                for j in range(G1):
                    ph1 = ps.tile([P, 272], F32, space="PSUM", tag="acc")
                    nc.tensor.matmul(out=ph1[:], lhsT=xg[:, j * P:(j + 1) * P],
                                     rhs=w1aug_s[:], start=True, stop=True)
                    dst = rows[:, j * 272:(j + 1) * 272]
                    if j % 2 == 0:
                        nc.scalar.copy(out=dst, in_=ph1[:])
                    else:
                        nc.vector.tensor_scalar_add(out=dst, in0=ph1[:],
                                                    scalar1=0.0)
                t1_writes.append(nc.sync.dma_start(
                    out=t1[B0 * P:(B0 + G1) * P, 0:272].rearrange(
                        "(j p) c -> p j c", j=G1),
                    in_=rows[:].rearrange("p (j c) -> p j c", j=G1)))

            j1tile = cpool.tile([1, 1], F32, tag="j1")
            j1 = nc.gpsimd.memset(j1tile[:], 0.0)
            for w in t1_writes:
                _dep(j1, w, "layer1 gathers wait for full node table")

            # ------------- phase 0b: own-node a_dst from t1 --------------
            # Per core all own rows sit in one table half; the other half's
            # call gathers only the sentinel row (idx data decides), and the
            # host-supplied sentinel a_dst is subtracted back out.
            MA = 4                               # block-cols per piece
            for k0 in range(0, BPC, MA):
                nk = min(MA, BPC - k0)
                bufs = []
                for half in range(2):
                    buf = p1pool.tile([P, MA * P], F16, tag=f"a1g{half}")
                    tslc = t1[SPLIT:NPAD, 256:384] if half \
                        else t1[0:SPLIT, 256:384]
                    q = _GQ[0] % 4
                    _GQ[0] += 1
                    g = nc.gpsimd.dma_gather(
                        out_ap=buf[:, 0:nk * P]
                            .rearrange("p (m w) -> p m w", m=nk),
                        in_ap=tslc,
                        idxs_ap=aidx_s[:, half * BPC * 8 + k0 * 8:
                                       half * BPC * 8 + (k0 + nk) * 8],
                        num_idxs=nk * P, num_idxs_reg=nk * P,
                        elem_size=P, elem_step=T1W, queue_num=q)
                    _dep(g, j1, "a_dst gather after t1")
                    bufs.append(buf)
                # adst cols are 8:16 of the gathered 128-col window
                asl = A1[:, k0 * H1:(k0 + nk) * H1]
                nc.vector.tensor_tensor(
                    out=asl.rearrange("p (b h) -> p b h", b=nk),
                    in0=bufs[0][:, 0:nk * P]
                        .rearrange("p (b w) -> p b w", b=nk)[:, :, 8:16],
                    in1=bufs[1][:, 0:nk * P]
                        .rearrange("p (b w) -> p b w", b=nk)[:, :, 8:16],
                    op=ALU.add)
                nc.vector.tensor_tensor(
                    out=asl.rearrange("p (b h) -> p b h", b=nk),
                    in0=asl.rearrange("p (b h) -> p b h", b=nk),
                    in1=sentadst_s[:].unsqueeze(1).to_broadcast([P, nk, H1]),
                    op=ALU.subtract)

            def tree_reduce(width, G, bufA, bufB):
                """Sum bufA[:, :G*width] over the G groups; ping-pongs
                between bufA and bufB. Returns the buffer whose cols
                [0:width] hold the sums."""
                src, dstb, n = bufA, bufB, G
                while n > 1:
                    h = n // 2
                    odd = n - 2 * h
                    nc.vector.tensor_tensor(
                        out=dstb[:, 0:h * width],
                        in0=src[:, 0:h * width],
                        in1=src[:, h * width:2 * h * width],
                        op=ALU.add)
                    if odd:
                        nc.vector.tensor_tensor(
                            out=dstb[:, 0:width],
                            in0=dstb[:, 0:width],
                            in1=src[:, 2 * h * width:(2 * h + 1) * width],
                            op=ALU.add)
                    src, dstb = dstb, src
                    n = h
                return src

            # ------------- phase 2: layer-1 per-dst aggregation ----------
            # Blocks are processed in column segments of <= GCAP so gather
            # tiles are small enough for a deep (bufs=4) pipeline; segment
            # partial sums accumulate into a per-block accumulator.
            W264 = HC1 + H1

            def segments(G):
                k = -(-G // GCAP)
                bounds = [2 * round(i * G / k / 2) for i in range(k + 1)]
                bounds[-1] = G
                return list(zip(bounds[:-1], bounds[1:]))

            def gather_seg(b, c0, c1, grow, tab, elem, dep, why):
                """Gather grid columns [c0, c1) of block b into grow."""
                glo = cfg.glo[b]
                ibase = 8 * sum(cfg.g[:b])
                nseg = c1 - c0
                eix = gpool.tile([P, GCAP * 8], I16, tag="eix")
                nc.sync.dma_start(
                    out=eix[:, 0:nseg * 8],
                    in_=eidx[:, ibase + c0 * 8:ibase + c1 * 8])
                ranges = []
                if c0 < glo:
                    ranges.append((c0, min(c1, glo), 0))
                if c1 > glo:
                    ranges.append((max(c0, glo), c1, 1))
                for (r0_, r1_, hi) in ranges:
                    tslc = tab[SPLIT:NPAD, 0:elem] if hi \
                        else tab[0:SPLIT, 0:elem]
                    for k0 in range(r0_, r1_, MAXCOLS):
                        nk = min(MAXCOLS, r1_ - k0)
                        q = _GQ[0] % 4
                        _GQ[0] += 1
                        g = nc.gpsimd.dma_gather(
                            out_ap=grow[:, (k0 - c0) * elem:
                                        (k0 - c0 + nk) * elem]
                                .rearrange("p (m w) -> p m w", m=nk),
                            in_ap=tslc,
                            idxs_ap=eix[:, (k0 - c0) * 8:(k0 - c0 + nk) * 8],
                            num_idxs=nk * 128, num_idxs_reg=nk * 128,
                            elem_size=elem, queue_num=q)
                        _dep(g, dep, why)

            cc_writes = []
            for b in range(BPC):
                G = cfg.g[b]
                r0 = b * P
                segs = segments(G)
                acc = opool.tile([P, W264], F16, tag="acc1")
                for si, (c0, c1) in enumerate(segs):
                    ncol = c1 - c0
                    grow = gpool.tile([P, GCAP * T1W], F16, tag="grow")
                    gather_seg(b, c0, c1, grow, t1, T1W, j1, "l1 gather")
                    grv = grow[:, :ncol * T1W].rearrange(
                        "p (g c) -> p g c", g=ncol)

                    av = bpool.tile([P, GCAP * H1], F16, tag="av")
                    nc.vector.tensor_tensor(
                        out=av[:, :ncol * H1].rearrange(
                            "p (g h) -> p g h", g=ncol),
                        in0=grv[:, :, 256:264],
                        in1=A1[:, b * H1:(b + 1) * H1].unsqueeze(1)
                            .to_broadcast([P, ncol, H1]),
                        op=ALU.add)
                    lk = bpool.tile([P, GCAP * H1], F16, tag="lk")
                    nc.vector.scalar_tensor_tensor(
                        out=lk[:, :ncol * H1], in0=av[:, :ncol * H1],
                        scalar=NEG_SLOPE, in1=av[:, :ncol * H1],
                        op0=ALU.mult, op1=ALU.max)

                    wm = bpool.tile([P, GCAP * W264], F16, tag="wm")
                    wmv = wm[:, :ncol * W264].rearrange(
                        "p (g c) -> p g c", g=ncol)
                    # exp, broadcast 8 heads -> 256 lanes, on the scalar
                    # engine, directly into wm's message columns
                    nc.scalar.activation(
                        out=wmv[:, :, 0:HC1].rearrange(
                            "p g (h c) -> p g h c", h=H1),
                        in_=lk[:, :ncol * H1].rearrange(
                            "p (g h) -> p g h", g=ncol)
                            .unsqueeze(3).to_broadcast([P, ncol, H1, CH1]),
                        func=AF.Exp, bias=shifts_s[:, 0:1])
                    # denominator columns: one e per head (stride-32 picks)
                    nc.vector.tensor_copy(
                        out=wmv[:, :, HC1:W264],
                        in_=wmv[:, :, 0:HC1].rearrange(
                            "p g (h c) -> p g h c", h=H1)[:, :, :, 0])
                    nc.vector.tensor_tensor(
                        out=wmv[:, :, 0:HC1],
                        in0=grv[:, :, 0:HC1],
                        in1=wmv[:, :, 0:HC1],
                        op=ALU.mult)

                    redt = bpool.tile([P, (GCAP // 2 + 1) * W264], F16,
                                      tag="redt")
                    red = tree_reduce(W264, ncol, wm, redt)
                    if si == 0:
                        nc.vector.tensor_copy(out=acc[:],
                                              in_=red[:, 0:W264])
                    else:
                        nc.vector.tensor_tensor(
                            out=acc[:], in0=acc[:], in1=red[:, 0:W264],
                            op=ALU.add)

                msum = acc[:, 0:HC1]
                dsum = acc[:, HC1:W264]
                denf = opool.tile([P, H1], F32, tag="denf")
                nc.vector.tensor_scalar_add(out=denf[:], in0=dsum,
                                            scalar1=EPS)
                rec = opool.tile([P, H1], F32, tag="rec")
                nc.vector.reciprocal(out=rec[:], in_=denf[:])
                recb = opool.tile([P, HC1], F16, tag="recb")
                nc.vector.tensor_scalar(
                    out=recb[:].rearrange("p (h c) -> p h c", h=H1),
                    in0=rec[:].unsqueeze(2).to_broadcast([P, H1, CH1]),
                    scalar1=60000.0, scalar2=None, op0=ALU.min)
                o1b = opool.tile([P, HC1], F16, tag="o1b")
                nc.vector.tensor_tensor(out=o1b[:], in0=msum, in1=recb[:],
                                        op=ALU.mult)
                nc.vector.tensor_tensor(out=o1b[:], in0=o1b[:], in1=b1b_s[:],
                                        op=ALU.add)
                xn = opool.tile([P, HC1], F16, tag="recb")
                nc.vector.tensor_scalar_min(out=xn[:], in0=o1b[:], scalar1=0.0)
                en = opool.tile([P, HC1], F16, tag="recb")
                nc.scalar.activation(out=en[:], in_=xn[:], func=AF.Exp)
                helu = opool.tile([P, HC1], F16, tag="o1b")
                nc.vector.scalar_tensor_tensor(
                    out=helu[:], in0=o1b[:], scalar=0.0, in1=en[:],
                    op0=ALU.max, op1=ALU.add)

                ph2 = ps.tile([P, 66], F32, space="PSUM", tag="ph2")
                for j in range(HC1 // P):
                    pT = ps.tile([P, P], F16, space="PSUM", tag="pT")
                    nc.tensor.transpose(out=pT[:],
                                        in_=helu[:, j * P:(j + 1) * P],
                                        identity=ident_s[:])
                    hT = opool.tile([P, P], F16, tag="hT")
                    nc.scalar.copy(out=hT[:], in_=pT[:])
                    nc.tensor.matmul(out=ph2[:], lhsT=hT[:], rhs=w2aug_s[j][:],
                                     start=(j == 0), stop=(j == HC1 // P - 1))
                h2row = opool.tile([P, 66], F16, tag="h2row")
                nc.scalar.copy(out=h2row[:, 0:66], in_=ph2[:])
                # force sentinel/pad-node layer-2 a_src very negative
                nc.vector.tensor_tensor(
                    out=h2row[:, 64:65], in0=h2row[:, 64:65],
                    in1=padmask_s[:, b:b + 1], op=ALU.add)
                nc.scalar.copy(out=A2[:, b:b + 1], in_=h2row[:, 65:66])
                cc_writes.append(nc.sync.dma_start(
                    out=cc_in[r0:r0 + P, 0:66], in_=h2row[:]))

            # ------------- phase 3: share layer-2 node table -------------
            cc = nc.gpsimd.collective_compute(
                "AllGather", ALU.bypass, replica_groups=groups,
                ins=[cc_in[:]], outs=[t2[:]])
            for w in cc_writes:
                _dep(cc, w, "allgather after cc writes")
            j2tile = cpool.tile([1, 1], F32, tag="j2")
            j2 = nc.gpsimd.memset(j2tile[:], 0.0)
            _dep(j2, cc, "layer2 gathers after allgather")

            # ------------- phase 4: layer-2 per-dst aggregation ----------
            for b in range(BPC):
                G = cfg.g[b]
                r0 = b * P
                segs = segments(G)
                acc2 = opool.tile([P, C2], F16, tag="acc2")
                acc2d = opool.tile([P, 2], F32, tag="acc2d")
                for si, (c0, c1) in enumerate(segs):
                    ncol = c1 - c0
                    grow2 = g2pool.tile([P, GCAP * T2W], F16, tag="grow2")
                    gather_seg(b, c0, c1, grow2, t2, T2W, j2, "l2 gather")
                    grv2 = grow2[:, :ncol * T2W].rearrange(
                        "p (g c) -> p g c", g=ncol)

                    av2 = bpool.tile([P, GCAP], F16, tag="av2")
                    nc.vector.scalar_tensor_tensor(
                        out=av2[:, :ncol],
                        in0=grv2[:, :, 64:65].rearrange("p g o -> p (g o)"),
                        scalar=shifts_s[:, 1:2],
                        in1=A2[:, b:b + 1].to_broadcast([P, ncol]),
                        op0=ALU.add, op1=ALU.add)
                    lk2 = bpool.tile([P, GCAP], F16, tag="lk2")
                    nc.vector.scalar_tensor_tensor(
                        out=lk2[:, :ncol], in0=av2[:, :ncol], scalar=NEG_SLOPE,
                        in1=av2[:, :ncol], op0=ALU.mult, op1=ALU.max)
                    e2 = bpool.tile([P, GCAP], F16, tag="e2")
                    nc.scalar.activation(out=e2[:, :ncol], in_=lk2[:, :ncol],
                                         func=AF.Exp, bias=shifts_s[:, 0:1])

                    wm2 = bpool.tile([P, GCAP * C2], F16, tag="wm")
                    wm2v = wm2[:, :ncol * C2].rearrange(
                        "p (g c) -> p g c", g=ncol)
                    nc.vector.tensor_tensor(
                        out=wm2v[:],
                        in0=grv2[:, :, 0:C2],
                        in1=e2[:, :ncol].unsqueeze(2)
                            .to_broadcast([P, ncol, C2]),
                        op=ALU.mult)

                    redt2 = bpool.tile([P, (GCAP // 2 + 1) * C2], F16,
                                       tag="redt")
                    red2 = tree_reduce(C2, ncol, wm2, redt2)
                    dpart = opool.tile([P, 2], F32, tag="dpart")
                    nc.vector.tensor_reduce(
                        out=dpart[:, 0:1], in_=e2[:, :ncol],
                        axis=mybir.AxisListType.X, op=ALU.add)
                    if si == 0:
                        nc.vector.tensor_copy(out=acc2[:], in_=red2[:, 0:C2])
                        nc.vector.tensor_copy(out=acc2d[:, 0:1],
                                              in_=dpart[:, 0:1])
                    else:
                        nc.vector.tensor_tensor(
                            out=acc2[:], in0=acc2[:], in1=red2[:, 0:C2],
                            op=ALU.add)
                        nc.vector.tensor_tensor(
                            out=acc2d[:, 0:1], in0=acc2d[:, 0:1],
                            in1=dpart[:, 0:1], op=ALU.add)

                den2f = opool.tile([P, 1], F32, tag="den2f")
                nc.vector.tensor_scalar_add(out=den2f[:],
                                            in0=acc2d[:, 0:1],
                                            scalar1=EPS)
                rec2 = opool.tile([P, 1], F32, tag="rec2")
                nc.vector.reciprocal(out=rec2[:], in_=den2f[:])
                o2 = opool.tile([P, C2], F32, tag="o2")
                nc.vector.scalar_tensor_tensor(
                    out=o2[:], in0=acc2[:], scalar=rec2[:, 0:1],
                    in1=b2b_s[:], op0=ALU.mult, op1=ALU.add)
                nc.sync.dma_start(out=out[r0:r0 + P, :], in_=o2[:])

    nc.compile()
    return nc


def _wrap16(idx):
    """Pack int16 idx array (len multiple of 128) into wrapped-16 layout
    [128, n//16]: element k at (k%16, k//16), replicated to rows 16..127."""
    n = len(idx)
    a = np.asarray(idx, np.int16).reshape(n // 16, 16).T  # [16, n//16]
    return np.tile(a, (8, 1))


def _deal_half(order, ranked_nodes, id0, half_cores, bpc):
    """Deal deg-sorted nodes of one class into its 4 cores' id range:
    consecutive 128-node blocks go to (core stripe, position) pairs."""
    nh = len(ranked_nodes)
    r = np.arange(nh)
    blk = r >> 7
    ids = (id0 + ((blk % half_cores) * bpc + blk // half_cores) * P
           + (r & 127))
    order[ids] = ranked_nodes


def compute_layout(n, edge_index):
    """Node permutation + per-position (Glo, Ghi) column counts."""
    bpc = -(-n // (P * NCORES))
    npad = NCORES * bpc * P
    nblk = NCORES * bpc
    half = npad // 2                 # == SPLIT for the 8-core layout
    assert half == SPLIT

    src = np.asarray(edge_index[0]).astype(np.int64)
    dst = np.asarray(edge_index[1]).astype(np.int64)
    deg = np.bincount(dst, minlength=n) + 1      # + self loop

    # fix each node's lo/hi CLASS up front (alternating deg-rank blocks
    # so both halves get identical degree profiles); classes never move,
    # so per-node lo-counts are exact, not a fixed-point guess.
    rank_of = np.argsort(-deg, kind="stable")    # rank -> orig node
    rank_inv = np.empty(n, np.int64)
    rank_inv[rank_of] = np.arange(n)
    is_lo = ((rank_inv >> 7) % NCORES) < (NCORES // 2)   # per orig node
    # keep >=1 pad id in each half for its sentinel row
    for flip_from, flag in ((is_lo, True), (~is_lo, False)):
        excess = int(flip_from.sum()) - (half - 1)
        if excess > 0:
            cand = np.nonzero(flip_from)[0]
            worst = cand[np.argsort(rank_inv[cand])[-excess:]]
            is_lo[worst] = not flag

    lo_cnt = np.bincount(dst[is_lo[src]], minlength=n)
    lo_cnt += is_lo                               # self loop
    hi_cnt = deg - lo_cnt
    key = lo_cnt.astype(np.int64) * (4 * npad) + hi_cnt
    krank = np.argsort(key, kind="stable")

    order = np.full(npad, -1, np.int64)           # new id -> orig node
    lo_nodes = krank[is_lo[krank]]
    hi_nodes = krank[~is_lo[krank]]
    assert len(lo_nodes) <= half and len(hi_nodes) <= half
    _deal_half(order, lo_nodes, 0, NCORES // 2, bpc)
    _deal_half(order, hi_nodes, half, NCORES // 2, bpc)

    # force sentinel ids (last row of each table half) to be pads
    for sent, lim0, lim1 in ((SPLIT - 1, 0, half), (npad - 1, half, npad)):
        if order[sent] >= 0:
            padq = np.nonzero(order[lim0:lim1] < 0)[0]
            assert len(padq), "no pad id available in half"
            q = lim0 + padq[-1]
            order[q] = order[sent]
            order[sent] = -1
    new_id = np.full(n, -1, np.int64)
    real = np.nonzero(order >= 0)[0]
    new_id[order[real]] = real
    # classes preserved by construction
    assert (new_id[lo_nodes] < SPLIT).all()
    assert (new_id[hi_nodes] >= SPLIT).all()

    # exact per-block lo/hi maxima under the final assignment
    src_n = new_id[src]
    dst_n = new_id[dst]
    lo_edge = src_n < SPLIT
    lo_c = np.bincount(dst_n[lo_edge], minlength=npad)
    hi_c = np.bincount(dst_n[~lo_edge], minlength=npad)
    sl = np.nonzero(order >= 0)[0]               # self loops (new ids)
    np.add.at(lo_c, sl[sl < SPLIT], 1)
    np.add.at(hi_c, sl[sl >= SPLIT], 1)

    lo_blk = lo_c.reshape(nblk, P).max(axis=1)
    hi_blk = hi_c.reshape(nblk, P).max(axis=1)
    glo = lo_blk.reshape(NCORES, bpc).max(axis=0)
    ghi = hi_blk.reshape(NCORES, bpc).max(axis=0)
    glo = glo.astype(np.int64)
    ghi = ghi.astype(np.int64)
    for i in range(bpc):
        if (glo[i] + ghi[i]) % 2:
            ghi[i] += 1
        if glo[i] + ghi[i] == 0:
            ghi[i] = 2
    return order, new_id, [int(v) for v in glo], [int(v) for v in ghi]


def host_prep(cfg, x, W1, att_src1, att_dst1, bias1, W2, att_src2,
              att_dst2, bias2, edge_index):
    n = cfg.n
    NPAD, BPC = cfg.npad, cfg.bpc
    H1, CH1, HC1 = cfg.h1, cfg.ch1, cfg.hc1
    order, new_id = cfg.order, cfg.new_id

    src = np.asarray(edge_index[0]).astype(np.int64)
    dst = np.asarray(edge_index[1]).astype(np.int64)
    loop = np.arange(n, dtype=np.int64)
    src = np.concatenate([src, loop])
    dst = np.concatenate([dst, loop])
    src_n = new_id[src]
    dst_n = new_id[dst]

    # group edges by new dst, lo-src first within each node
    lo_flag = src_n < SPLIT
    eorder = np.argsort(dst_n * 2 + (~lo_flag), kind="stable")
    src_s = src_n[eorder]
    dst_s = dst_n[eorder]
    lo_s = lo_flag[eorder]

    counts = np.bincount(dst_s, minlength=NPAD)
    lo_cnt = np.bincount(dst_s[lo_s], minlength=NPAD)
    starts = np.zeros(NPAD + 1, np.int64)
    np.cumsum(counts, out=starts[1:])
    pos_in_node = np.arange(len(src_s)) - starts[dst_s]

    SENT_LO = SPLIT - 1
    SENT_HI_REL = (NPAD - 1) - SPLIT
    blk_of = dst_s >> 7
    part_of = dst_s & 127

    SG8 = 8 * cfg.sg
    eidx_cores = [np.empty((P, SG8), np.int16) for _ in range(NCORES)]
    ibase = 0
    for bpos in range(BPC):
        glo, ghi = cfg.glo[bpos], cfg.ghi[bpos]
        G = glo + ghi
        for c in range(NCORES):
            blk = c * BPC + bpos
            grid = np.empty((P, G), np.int16)
            grid[:, :glo] = SENT_LO
            grid[:, glo:] = SENT_HI_REL
            m = blk_of == blk
            pp = part_of[m]
            sv = src_s[m]
            lv = lo_s[m]
            pn = pos_in_node[m]
            ln = lo_cnt[blk * P + pp]
            col = np.where(lv, pn, glo + (pn - ln))
            grid[pp, col] = np.where(lv, sv, sv - SPLIT).astype(np.int16)
            flat = grid.T.reshape(-1)            # k = g*128 + p
            eidx_cores[c][:, ibase:ibase + G * 8] = _wrap16(flat)
        ibase += G * 8

    # ---- parameter prep ----
    x = np.asarray(x, np.float32)
    W1 = np.asarray(W1, np.float32)
    W2 = np.asarray(W2, np.float32)
    as1 = np.asarray(att_src1, np.float32)
    ad1 = np.asarray(att_dst1, np.float32)
    as2 = np.asarray(att_src2, np.float32).reshape(-1)
    ad2 = np.asarray(att_dst2, np.float32).reshape(-1)

    A1s = np.zeros((HC1, H1), dtype=np.float32)
    A1d = np.zeros((HC1, H1), dtype=np.float32)
    hh = np.repeat(np.arange(H1), CH1)
    A1s[np.arange(HC1), hh] = as1.reshape(-1)
    A1d[np.arange(HC1), hh] = ad1.reshape(-1)
    Bs = W1 @ A1s                                   # [c_in, H1]
    # sentinel x: a_src(x_sent) == -100 per head, minimal norm
    x_sent = Bs @ np.linalg.solve(Bs.T @ Bs, -100.0 * np.ones(H1))

    xp = np.empty((NPAD, cfg.c_in), np.float32)
    real = order >= 0
    xp[real] = x[order[real]]
    xp[~real] = x_sent
    xt = np.ascontiguousarray(xp.T).astype(np.float16)

    w1aug = np.concatenate([W1, Bs, W1 @ A1d], axis=1).astype(np.float16)
    w2aug = np.concatenate([W2, (W2 @ as2)[:, None], (W2 @ ad2)[:, None]],
                           axis=1).astype(np.float16)

    colsum = W2.sum(axis=0)
    c0 = float(colsum @ (as2 + ad2))
    shifts = np.zeros((P, 2), dtype=np.float32)
    shifts[:, 0] = -2.0
    shifts[:, 1] = -c0

    b1b = np.tile(np.asarray(bias1, np.float32).reshape(1, -1),
                  (P, 1)).astype(np.float16)
    b2b = np.tile((np.asarray(bias2, np.float32).reshape(-1) - colsum
                   ).reshape(1, -1), (P, 1)).astype(np.float32)
    ident = np.eye(P, dtype=np.float16)

    # sentinel a_dst as the device computes it (fp16 inputs, fp32 matmul)
    sentadst_v = (x_sent.astype(np.float16).astype(np.float32)
                  @ w1aug[:, 264:272].astype(np.float32)).astype(np.float16)
    sentadst = np.tile(sentadst_v.reshape(1, -1), (P, 1))

    SENT_HI = (NPAD - 1) - SPLIT
    in_maps = []
    for c in range(NCORES):
        base = c * BPC * P
        pm = np.zeros((P, BPC), np.float16)
        ids = (base + np.arange(BPC)[None, :] * P +
               np.arange(P)[:, None])
        pm[:, :] = np.where(order[ids] < 0, -1000.0, 0.0)
        # own-row a_dst gather indices (lo call | hi call)
        own = base + np.arange(BPC * P, dtype=np.int64)
        if base + BPC * P <= SPLIT:
            alo, ahi = own, np.full(BPC * P, SENT_HI, np.int64)
        else:
            alo = np.full(BPC * P, SPLIT - 1, np.int64)
            ahi = own - SPLIT
        aidxv = np.concatenate(
            [_wrap16(alo.astype(np.int16)), _wrap16(ahi.astype(np.int16))],
            axis=1)
        in_maps.append({
            "xt": xt, "w1aug": w1aug, "w2aug": w2aug, "b1b": b1b,
            "b2b": b2b, "shifts": shifts, "ident": ident,
            "padmask": pm, "eidx": eidx_cores[c], "aidx": aidxv,
            "sentadst": sentadst})
    return in_maps


_prog_cache = {}
_last_results = None


def kernel(x, edge_index, edge_weight, W1, att_src1, att_dst1, bias1,
           W2, att_src2, att_dst2, bias2):
    global _last_results
    n = x.shape[0]
    # edge_weight is unused (GATConv with edge_dim=None ignores it)
    order, new_id, glo, ghi = compute_layout(n, edge_index)
    cfg = Cfg(n, glo, ghi, c_in=x.shape[1])
    cfg.order, cfg.new_id = order, new_id
    key = (cfg.n, cfg.c_in, tuple(glo), tuple(ghi))
    if key not in _prog_cache:
        _prog_cache[key] = build_program(cfg)
    nc = _prog_cache[key]

    in_maps = host_prep(cfg, x, W1, att_src1, att_dst1, bias1, W2,
                        att_src2, att_dst2, bias2, edge_index)
    res = run_bass_kernel_spmd(nc, in_maps, list(range(cfg.ncores)))
    _last_results = res
    full = np.concatenate([res.results[c]["out"]
                           for c in range(cfg.ncores)], axis=0)
    out = np.zeros((n, cfg.c2), np.float32)
    real = order >= 0
    out[order[real]] = full[real]
    return np.ascontiguousarray(out)


# revision 35
# speedup vs baseline: 1.0487x; 1.0487x over previous
"""GAT (2-layer, PyG-style) Trainium2 Bass kernel, 8-core SPMD. v4.

Strategy (dst-per-partition, reduction-based aggregation):
- Host renumbers nodes: sort by (degree desc, lo-count), deal rank-blocks
  of 128 to (core, position) so all 8 cores' blocks at the same position
  have near-equal max degree.  Core c owns contiguous new-ids
  [c*BPC*128, (c+1)*BPC*128).  Partition p of block b IS dst node; its
  edges lie along the free dim as [lo-src | pad | hi-src | pad], padded
  to per-position (Glo, Ghi) maxima over cores (the compiled program is
  shared by all cores).
- a_dst is a per-partition scalar (block-local SBUF table from phase 0)
  => no a_dst gather, no one-hot scatter matmuls.  Segment softmax +
  scatter-add become per-partition row ops + a free-dim tree reduction.
- h/a_src rows are fetched with InstDMAGatherAnt (int16 idx) from node
  tables split at row 25088 so both halves' indices fit int16. Pad slots
  gather a sentinel row whose a_src is -100 => exp()==0 in fp16, so
  padding contributes exactly zero to message and denominator sums.
- Logits are exp-shifted by -2 (cancels in softmax; keeps fp16 finite).
  ELU's "-1" is dropped (eluplus = relu(x)+exp(min(x,0))) and corrected
  at the end: out -= colsum(W2); the induced constant layer-2 logit
  shift c0 is subtracted pre-leaky_relu.  Sentinel layer-2 a_src is
  forced to -1000 via a padmask input baked in before the AllGather.
- One AllGather of the 128-col fp16 layer-2 node table is the only
  collective.
"""

import numpy as np

import concourse.bacc as bacc
import concourse.mybir as mybir
import concourse.tile as tile
from concourse.bass_utils import run_bass_kernel_spmd
from bass_rust import add_dep_helper


def _dep(a, b, reason):
    ia = a.ins if hasattr(a, "ins") else a
    ib = b.ins if hasattr(b, "ins") else b
    add_dep_helper(ia, ib, reason=reason)


P = 128
NCORES = 8
EPS = 1e-16
NEG_SLOPE = 0.2
SPLIT = 25088               # node-table split so gather idx fits int16
F32 = mybir.dt.float32
F16 = mybir.dt.float16
I32 = mybir.dt.int32
I16 = mybir.dt.int16
AF = mybir.ActivationFunctionType
ALU = mybir.AluOpType

MAXCOLS = 8                 # <=8 cols (1024 idx) per gather call
GCAP = 24                   # max grid columns per processing segment


class Cfg:
    def __init__(self, n_nodes, glo, ghi, c_in=128, h1=8, ch1=32, c2=64,
                 ncores=NCORES):
        self.n = n_nodes
        self.c_in = c_in
        self.h1 = h1
        self.ch1 = ch1
        self.hc1 = h1 * ch1          # 256
        self.c2 = c2
        self.ncores = ncores
        self.bpc = -(-n_nodes // (P * ncores))      # 49
        self.npad = ncores * self.bpc * P
        self.nblk = ncores * self.bpc
        self.t1w = 384               # [h 256 | asrc 8 | adst 8 | junk]
        self.t2w = 128               # [h2 64 | asrc2 1 | adst2 1 | junk]
        self.glo = list(glo)         # per block position, len bpc
        self.ghi = list(ghi)
        self.g = [a + b for a, b in zip(self.glo, self.ghi)]
        self.sg = sum(self.g)
        self.gmax = max(self.g)


_GQ = [0]


def build_program(cfg):
    nc = bacc.Bacc(None, num_devices=cfg.ncores, num_swdge_queues=4,
                   dynamic_dma_scratch_size=16384)
    HC1, H1, CH1, C2 = cfg.hc1, cfg.h1, cfg.ch1, cfg.c2
    T1W, T2W, BPC = cfg.t1w, cfg.t2w, cfg.bpc
    NBLK, NPAD = cfg.nblk, cfg.npad
    G1 = 8
    G0 = -(-BPC // G1)              # phase-0 groups of 8 blocks

    # ---- I/O ----
    xt = nc.dram_tensor("xt", [cfg.c_in, NPAD], F16, kind="ExternalInput")
    w1aug = nc.dram_tensor("w1aug", [cfg.c_in, 272], F16, kind="ExternalInput")
    w2aug = nc.dram_tensor("w2aug", [HC1, 66], F16, kind="ExternalInput")
    b1b = nc.dram_tensor("b1b", [P, HC1], F16, kind="ExternalInput")
    b2b = nc.dram_tensor("b2b", [P, C2], F32, kind="ExternalInput")
    shifts = nc.dram_tensor("shifts", [P, 2], F32, kind="ExternalInput")
    ident = nc.dram_tensor("ident", [P, P], F16, kind="ExternalInput")
    padmask = nc.dram_tensor("padmask", [P, BPC], F16, kind="ExternalInput")
    eidx = nc.dram_tensor("eidx", [P, 8 * cfg.sg], I16, kind="ExternalInput")
    aidx = nc.dram_tensor("aidx", [P, 2 * BPC * 8], I16, kind="ExternalInput")
    sentadst = nc.dram_tensor("sentadst", [P, H1], F16, kind="ExternalInput")
    out = nc.dram_tensor("out", [BPC * P, C2], F32, kind="ExternalOutput")

    # ---- internal DRAM ----
    t1 = nc.dram_tensor("t1", [NPAD, T1W], F16)
    cc_in = nc.dram_tensor("cc_in", [BPC * P, T2W], F16)
    t2 = nc.dram_tensor("t2", [NPAD, T2W], F16, addr_space="Shared")

    groups = [list(range(cfg.ncores))]
    GM = cfg.gmax

    with tile.TileContext(nc) as tc:
        with (
            tc.tile_pool(name="const", bufs=1) as cpool,
            tc.tile_pool(name="p1", bufs=2) as p1pool,
            tc.tile_pool(name="gat", bufs=5) as gpool,
            tc.tile_pool(name="blk", bufs=3) as bpool,
            tc.tile_pool(name="fin", bufs=4) as opool,
            tc.tile_pool(name="ps", bufs=2, space="PSUM") as ps,
        ):
            # ---------------- constants ----------------
            ident_s = cpool.tile([P, P], F16)
            nc.sync.dma_start(out=ident_s[:], in_=ident[:])
            w1aug_s = cpool.tile([P, 272], F16)
            nc.sync.dma_start(out=w1aug_s[:], in_=w1aug[:])
            w2aug_s = []
            for j in range(HC1 // P):
                wg = cpool.tile([P, 66], F16, tag=f"w2aug{j}")
                nc.sync.dma_start(out=wg[:], in_=w2aug[j * P:(j + 1) * P, :])
                w2aug_s.append(wg)
            b1b_s = cpool.tile([P, HC1], F16)
            nc.sync.dma_start(out=b1b_s[:], in_=b1b[:])
            b2b_s = cpool.tile([P, C2], F32)
            nc.sync.dma_start(out=b2b_s[:], in_=b2b[:])
            shifts_s = cpool.tile([P, 2], F32)
            nc.sync.dma_start(out=shifts_s[:], in_=shifts[:])
            padmask_s = cpool.tile([P, BPC], F16)
            nc.sync.dma_start(out=padmask_s[:], in_=padmask[:])
            aidx_s = cpool.tile([P, 2 * BPC * 8], I16)
            nc.sync.dma_start(out=aidx_s[:], in_=aidx[:])
            sentadst_s = cpool.tile([P, H1], F16)
            nc.sync.dma_start(out=sentadst_s[:], in_=sentadst[:])
            # persistent per-core tables
            A1 = cpool.tile([P, BPC * H1], F16)       # own-node a_dst
            A2 = cpool.tile([P, BPC], F16)            # own-node a_dst2

            # ------------- phase 1: full node table (replicated) ---------
            t1_writes = []
            for grp in range(NBLK // G1):
                B0 = grp * G1
                xg = p1pool.tile([P, G1 * P], F16, tag="xg")
                nc.sync.dma_start(out=xg[:], in_=xt[:, B0 * P:(B0 + G1) * P])
                rows = p1pool.tile([P, G1 * 272], F16, tag="rows")
                for j in range(G1):
                    ph1 = ps.tile([P, 272], F32, space="PSUM", tag="acc")
                    nc.tensor.matmul(out=ph1[:], lhsT=xg[:, j * P:(j + 1) * P],
                                     rhs=w1aug_s[:], start=True, stop=True)
                    dst = rows[:, j * 272:(j + 1) * 272]
                    if j % 2 == 0:
                        nc.scalar.copy(out=dst, in_=ph1[:])
                    else:
                        nc.vector.tensor_scalar_add(out=dst, in0=ph1[:],
                                                    scalar1=0.0)
                t1_writes.append(nc.sync.dma_start(
                    out=t1[B0 * P:(B0 + G1) * P, 0:272].rearrange(
                        "(j p) c -> p j c", j=G1),
                    in_=rows[:].rearrange("p (j c) -> p j c", j=G1)))

            j1tile = cpool.tile([1, 1], F32, tag="j1")
            j1 = nc.gpsimd.memset(j1tile[:], 0.0)
            for w in t1_writes:
                _dep(j1, w, "layer1 gathers wait for full node table")

            # ------------- phase 0b: own-node a_dst from t1 --------------
            # Per core all own rows sit in one table half; the other half's
            # call gathers only the sentinel row (idx data decides), and the
            # host-supplied sentinel a_dst is subtracted back out.
            MA = 8                               # block-cols per piece
            for k0 in range(0, BPC, MA):
                nk = min(MA, BPC - k0)
                bufs = []
                for half in range(2):
                    buf = p1pool.tile([P, MA * P], F16, tag=f"a1g{half}")
                    tslc = t1[SPLIT:NPAD, 256:384] if half \
                        else t1[0:SPLIT, 256:384]
                    q = _GQ[0] % 4
                    _GQ[0] += 1
                    g = nc.gpsimd.dma_gather(
                        out_ap=buf[:, 0:nk * P]
                            .rearrange("p (m w) -> p m w", m=nk),
                        in_ap=tslc,
                        idxs_ap=aidx_s[:, half * BPC * 8 + k0 * 8:
                                       half * BPC * 8 + (k0 + nk) * 8],
                        num_idxs=nk * P, num_idxs_reg=nk * P,
                        elem_size=P, elem_step=T1W, queue_num=q)
                    _dep(g, j1, "a_dst gather after t1")
                    bufs.append(buf)
                # adst cols are 8:16 of the gathered 128-col window
                asl = A1[:, k0 * H1:(k0 + nk) * H1]
                nc.vector.tensor_tensor(
                    out=asl.rearrange("p (b h) -> p b h", b=nk),
                    in0=bufs[0][:, 0:nk * P]
                        .rearrange("p (b w) -> p b w", b=nk)[:, :, 8:16],
                    in1=bufs[1][:, 0:nk * P]
                        .rearrange("p (b w) -> p b w", b=nk)[:, :, 8:16],
                    op=ALU.add)
                nc.vector.tensor_tensor(
                    out=asl.rearrange("p (b h) -> p b h", b=nk),
                    in0=asl.rearrange("p (b h) -> p b h", b=nk),
                    in1=sentadst_s[:].unsqueeze(1).to_broadcast([P, nk, H1]),
                    op=ALU.subtract)

            def tree_reduce(width, G, bufA, bufB):
                """Sum bufA[:, :G*width] over the G groups; ping-pongs
                between bufA and bufB. Returns the buffer whose cols
                [0:width] hold the sums."""
                src, dstb, n = bufA, bufB, G
                while n > 1:
                    h = n // 2
                    odd = n - 2 * h
                    nc.vector.tensor_tensor(
                        out=dstb[:, 0:h * width],
                        in0=src[:, 0:h * width],
                        in1=src[:, h * width:2 * h * width],
                        op=ALU.add)
                    if odd:
                        nc.vector.tensor_tensor(
                            out=dstb[:, 0:width],
                            in0=dstb[:, 0:width],
                            in1=src[:, 2 * h * width:(2 * h + 1) * width],
                            op=ALU.add)
                    src, dstb = dstb, src
                    n = h
                return src

            # ------------- phase 2: layer-1 per-dst aggregation ----------
            # Blocks are processed in column segments of <= GCAP so gather
            # tiles are small enough for a deep (bufs=4) pipeline; segment
            # partial sums accumulate into a per-block accumulator.
            W264 = HC1 + H1

            def segments(G):
                k = -(-G // GCAP)
                bounds = [2 * round(i * G / k / 2) for i in range(k + 1)]
                bounds[-1] = G
                return list(zip(bounds[:-1], bounds[1:]))

            def gather_seg(b, c0, c1, grow, tab, elem, dep, why):
                """Gather grid columns [c0, c1) of block b into grow."""
                glo = cfg.glo[b]
                ibase = 8 * sum(cfg.g[:b])
                nseg = c1 - c0
                eix = gpool.tile([P, GCAP * 8], I16, tag="eix")
                nc.sync.dma_start(
                    out=eix[:, 0:nseg * 8],
                    in_=eidx[:, ibase + c0 * 8:ibase + c1 * 8])
                ranges = []
                if c0 < glo:
                    ranges.append((c0, min(c1, glo), 0))
                if c1 > glo:
                    ranges.append((max(c0, glo), c1, 1))
                for (r0_, r1_, hi) in ranges:
                    tslc = tab[SPLIT:NPAD, 0:elem] if hi \
                        else tab[0:SPLIT, 0:elem]
                    for k0 in range(r0_, r1_, MAXCOLS):
                        nk = min(MAXCOLS, r1_ - k0)
                        q = _GQ[0] % 4
                        _GQ[0] += 1
                        g = nc.gpsimd.dma_gather(
                            out_ap=grow[:, (k0 - c0) * elem:
                                        (k0 - c0 + nk) * elem]
                                .rearrange("p (m w) -> p m w", m=nk),
                            in_ap=tslc,
                            idxs_ap=eix[:, (k0 - c0) * 8:(k0 - c0 + nk) * 8],
                            num_idxs=nk * 128, num_idxs_reg=nk * 128,
                            elem_size=elem, queue_num=q)
                        _dep(g, dep, why)

            cc_writes = []
            for b in range(BPC):
                G = cfg.g[b]
                r0 = b * P
                segs = segments(G)
                acc = opool.tile([P, W264], F16, tag="acc1")
                for si, (c0, c1) in enumerate(segs):
                    ncol = c1 - c0
                    grow = gpool.tile([P, GCAP * T1W], F16, tag="grow")
                    gather_seg(b, c0, c1, grow, t1, T1W, j1, "l1 gather")
                    grv = grow[:, :ncol * T1W].rearrange(
                        "p (g c) -> p g c", g=ncol)

                    av = bpool.tile([P, GCAP * H1], F16, tag="av")
                    nc.vector.tensor_tensor(
                        out=av[:, :ncol * H1].rearrange(
                            "p (g h) -> p g h", g=ncol),
                        in0=grv[:, :, 256:264],
                        in1=A1[:, b * H1:(b + 1) * H1].unsqueeze(1)
                            .to_broadcast([P, ncol, H1]),
                        op=ALU.add)
                    lk = bpool.tile([P, GCAP * H1], F16, tag="lk")
                    nc.vector.scalar_tensor_tensor(
                        out=lk[:, :ncol * H1], in0=av[:, :ncol * H1],
                        scalar=NEG_SLOPE, in1=av[:, :ncol * H1],
                        op0=ALU.mult, op1=ALU.max)

                    wm = bpool.tile([P, GCAP * W264], F16, tag="wm")
                    wmv = wm[:, :ncol * W264].rearrange(
                        "p (g c) -> p g c", g=ncol)
                    # exp, broadcast 8 heads -> 256 lanes, on the scalar
                    # engine, directly into wm's message columns
                    nc.scalar.activation(
                        out=wmv[:, :, 0:HC1].rearrange(
                            "p g (h c) -> p g h c", h=H1),
                        in_=lk[:, :ncol * H1].rearrange(
                            "p (g h) -> p g h", g=ncol)
                            .unsqueeze(3).to_broadcast([P, ncol, H1, CH1]),
                        func=AF.Exp, bias=shifts_s[:, 0:1])
                    # denominator columns: one e per head (stride-32 picks)
                    nc.vector.tensor_copy(
                        out=wmv[:, :, HC1:W264],
                        in_=wmv[:, :, 0:HC1].rearrange(
                            "p g (h c) -> p g h c", h=H1)[:, :, :, 0])
                    nc.vector.tensor_tensor(
                        out=wmv[:, :, 0:HC1],
                        in0=grv[:, :, 0:HC1],
                        in1=wmv[:, :, 0:HC1],
                        op=ALU.mult)

                    redt = bpool.tile([P, (GCAP // 2 + 1) * W264], F16,
                                      tag="redt")
                    red = tree_reduce(W264, ncol, wm, redt)
                    if si == 0:
                        nc.vector.tensor_copy(out=acc[:],
                                              in_=red[:, 0:W264])
                    else:
                        nc.vector.tensor_tensor(
                            out=acc[:], in0=acc[:], in1=red[:, 0:W264],
                            op=ALU.add)

                msum = acc[:, 0:HC1]
                dsum = acc[:, HC1:W264]
                denf = opool.tile([P, H1], F32, tag="denf")
                nc.vector.tensor_scalar_add(out=denf[:], in0=dsum,
                                            scalar1=EPS)
                rec = opool.tile([P, H1], F32, tag="rec")
                nc.vector.reciprocal(out=rec[:], in_=denf[:])
                recb = opool.tile([P, HC1], F16, tag="recb")
                nc.vector.tensor_scalar(
                    out=recb[:].rearrange("p (h c) -> p h c", h=H1),
                    in0=rec[:].unsqueeze(2).to_broadcast([P, H1, CH1]),
                    scalar1=60000.0, scalar2=None, op0=ALU.min)
                o1b = opool.tile([P, HC1], F16, tag="o1b")
                nc.vector.tensor_tensor(out=o1b[:], in0=msum, in1=recb[:],
                                        op=ALU.mult)
                nc.vector.tensor_tensor(out=o1b[:], in0=o1b[:], in1=b1b_s[:],
                                        op=ALU.add)
                xn = opool.tile([P, HC1], F16, tag="recb")
                nc.vector.tensor_scalar_min(out=xn[:], in0=o1b[:], scalar1=0.0)
                en = opool.tile([P, HC1], F16, tag="recb")
                nc.scalar.activation(out=en[:], in_=xn[:], func=AF.Exp)
                helu = opool.tile([P, HC1], F16, tag="o1b")
                nc.vector.scalar_tensor_tensor(
                    out=helu[:], in0=o1b[:], scalar=0.0, in1=en[:],
                    op0=ALU.max, op1=ALU.add)

                ph2 = ps.tile([P, 66], F32, space="PSUM", tag="ph2")
                for j in range(HC1 // P):
                    pT = ps.tile([P, P], F16, space="PSUM", tag="pT")
                    nc.tensor.transpose(out=pT[:],
                                        in_=helu[:, j * P:(j + 1) * P],
                                        identity=ident_s[:])
                    hT = opool.tile([P, P], F16, tag="hT")
                    nc.scalar.copy(out=hT[:], in_=pT[:])
                    nc.tensor.matmul(out=ph2[:], lhsT=hT[:], rhs=w2aug_s[j][:],
                                     start=(j == 0), stop=(j == HC1 // P - 1))
                h2row = opool.tile([P, 66], F16, tag="h2row")
                nc.scalar.copy(out=h2row[:, 0:66], in_=ph2[:])
                # force sentinel/pad-node layer-2 a_src very negative
                nc.vector.tensor_tensor(
                    out=h2row[:, 64:65], in0=h2row[:, 64:65],
                    in1=padmask_s[:, b:b + 1], op=ALU.add)
                nc.scalar.copy(out=A2[:, b:b + 1], in_=h2row[:, 65:66])
                cc_writes.append(nc.sync.dma_start(
                    out=cc_in[r0:r0 + P, 0:66], in_=h2row[:]))

            # ------------- phase 3: share layer-2 node table -------------
            cc = nc.gpsimd.collective_compute(
                "AllGather", ALU.bypass, replica_groups=groups,
                ins=[cc_in[:]], outs=[t2[:]])
            for w in cc_writes:
                _dep(cc, w, "allgather after cc writes")
            j2tile = cpool.tile([1, 1], F32, tag="j2")
            j2 = nc.gpsimd.memset(j2tile[:], 0.0)
            _dep(j2, cc, "layer2 gathers after allgather")

            # ------------- phase 4: layer-2 per-dst aggregation ----------
            for b in range(BPC):
                G = cfg.g[b]
                r0 = b * P
                segs = segments(G)
                acc2 = opool.tile([P, C2], F16, tag="acc2")
                acc2d = opool.tile([P, 2], F32, tag="acc2d")
                for si, (c0, c1) in enumerate(segs):
                    ncol = c1 - c0
                    grow2 = gpool.tile([P, GCAP * T1W], F16, tag="grow")
                    gather_seg(b, c0, c1, grow2, t2, T2W, j2, "l2 gather")
                    grv2 = grow2[:, :ncol * T2W].rearrange(
                        "p (g c) -> p g c", g=ncol)

                    av2 = bpool.tile([P, GCAP], F16, tag="av2")
                    nc.vector.scalar_tensor_tensor(
                        out=av2[:, :ncol],
                        in0=grv2[:, :, 64:65].rearrange("p g o -> p (g o)"),
                        scalar=shifts_s[:, 1:2],
                        in1=A2[:, b:b + 1].to_broadcast([P, ncol]),
                        op0=ALU.add, op1=ALU.add)
                    lk2 = bpool.tile([P, GCAP], F16, tag="lk2")
                    nc.vector.scalar_tensor_tensor(
                        out=lk2[:, :ncol], in0=av2[:, :ncol], scalar=NEG_SLOPE,
                        in1=av2[:, :ncol], op0=ALU.mult, op1=ALU.max)
                    e2 = bpool.tile([P, GCAP], F16, tag="e2")
                    nc.scalar.activation(out=e2[:, :ncol], in_=lk2[:, :ncol],
                                         func=AF.Exp, bias=shifts_s[:, 0:1])

                    wm2 = bpool.tile([P, GCAP * C2], F16, tag="wm")
                    wm2v = wm2[:, :ncol * C2].rearrange(
                        "p (g c) -> p g c", g=ncol)
                    nc.vector.tensor_tensor(
                        out=wm2v[:],
                        in0=grv2[:, :, 0:C2],
                        in1=e2[:, :ncol].unsqueeze(2)
                            .to_broadcast([P, ncol, C2]),
                        op=ALU.mult)

                    redt2 = bpool.tile([P, (GCAP // 2 + 1) * C2], F16,
                                       tag="redt")
                    red2 = tree_reduce(C2, ncol, wm2, redt2)
                    dpart = opool.tile([P, 2], F32, tag="dpart")
                    nc.vector.tensor_reduce(
                        out=dpart[:, 0:1], in_=e2[:, :ncol],
                        axis=mybir.AxisListType.X, op=ALU.add)
                    if si == 0:
                        nc.vector.tensor_copy(out=acc2[:], in_=red2[:, 0:C2])
                        nc.vector.tensor_copy(out=acc2d[:, 0:1],
                                              in_=dpart[:, 0:1])
                    else:
                        nc.vector.tensor_tensor(
                            out=acc2[:], in0=acc2[:], in1=red2[:, 0:C2],
                            op=ALU.add)
                        nc.vector.tensor_tensor(
                            out=acc2d[:, 0:1], in0=acc2d[:, 0:1],
                            in1=dpart[:, 0:1], op=ALU.add)

                den2f = opool.tile([P, 1], F32, tag="den2f")
                nc.vector.tensor_scalar_add(out=den2f[:],
                                            in0=acc2d[:, 0:1],
                                            scalar1=EPS)
                rec2 = opool.tile([P, 1], F32, tag="rec2")
                nc.vector.reciprocal(out=rec2[:], in_=den2f[:])
                o2 = opool.tile([P, C2], F32, tag="o2")
                nc.vector.scalar_tensor_tensor(
                    out=o2[:], in0=acc2[:], scalar=rec2[:, 0:1],
                    in1=b2b_s[:], op0=ALU.mult, op1=ALU.add)
                nc.sync.dma_start(out=out[r0:r0 + P, :], in_=o2[:])

    nc.compile()
    return nc


def _wrap16(idx):
    """Pack int16 idx array (len multiple of 128) into wrapped-16 layout
    [128, n//16]: element k at (k%16, k//16), replicated to rows 16..127."""
    n = len(idx)
    a = np.asarray(idx, np.int16).reshape(n // 16, 16).T  # [16, n//16]
    return np.tile(a, (8, 1))


def _deal_half(order, ranked_nodes, id0, half_cores, bpc):
    """Deal deg-sorted nodes of one class into its 4 cores' id range:
    consecutive 128-node blocks go to (core stripe, position) pairs."""
    nh = len(ranked_nodes)
    r = np.arange(nh)
    blk = r >> 7
    ids = (id0 + ((blk % half_cores) * bpc + blk // half_cores) * P
           + (r & 127))
    order[ids] = ranked_nodes


def compute_layout(n, edge_index):
    """Node permutation + per-position (Glo, Ghi) column counts."""
    bpc = -(-n // (P * NCORES))
    npad = NCORES * bpc * P
    nblk = NCORES * bpc
    half = npad // 2                 # == SPLIT for the 8-core layout
    assert half == SPLIT

    src = np.asarray(edge_index[0]).astype(np.int64)
    dst = np.asarray(edge_index[1]).astype(np.int64)
    deg = np.bincount(dst, minlength=n) + 1      # + self loop

    # fix each node's lo/hi CLASS up front (alternating deg-rank blocks
    # so both halves get identical degree profiles); classes never move,
    # so per-node lo-counts are exact, not a fixed-point guess.
    rank_of = np.argsort(-deg, kind="stable")    # rank -> orig node
    rank_inv = np.empty(n, np.int64)
    rank_inv[rank_of] = np.arange(n)
    is_lo = ((rank_inv >> 7) % NCORES) < (NCORES // 2)   # per orig node
    # keep >=1 pad id in each half for its sentinel row
    for flip_from, flag in ((is_lo, True), (~is_lo, False)):
        excess = int(flip_from.sum()) - (half - 1)
        if excess > 0:
            cand = np.nonzero(flip_from)[0]
            worst = cand[np.argsort(rank_inv[cand])[-excess:]]
            is_lo[worst] = not flag

    lo_cnt = np.bincount(dst[is_lo[src]], minlength=n)
    lo_cnt += is_lo                               # self loop
    hi_cnt = deg - lo_cnt
    key = lo_cnt.astype(np.int64) * (4 * npad) + hi_cnt
    krank = np.argsort(key, kind="stable")

    order = np.full(npad, -1, np.int64)           # new id -> orig node
    lo_nodes = krank[is_lo[krank]]
    hi_nodes = krank[~is_lo[krank]]
    assert len(lo_nodes) <= half and len(hi_nodes) <= half
    _deal_half(order, lo_nodes, 0, NCORES // 2, bpc)
    _deal_half(order, hi_nodes, half, NCORES // 2, bpc)

    # force sentinel ids (last row of each table half) to be pads
    for sent, lim0, lim1 in ((SPLIT - 1, 0, half), (npad - 1, half, npad)):
        if order[sent] >= 0:
            padq = np.nonzero(order[lim0:lim1] < 0)[0]
            assert len(padq), "no pad id available in half"
            q = lim0 + padq[-1]
            order[q] = order[sent]
            order[sent] = -1
    new_id = np.full(n, -1, np.int64)
    real = np.nonzero(order >= 0)[0]
    new_id[order[real]] = real
    # classes preserved by construction
    assert (new_id[lo_nodes] < SPLIT).all()
    assert (new_id[hi_nodes] >= SPLIT).all()

    # exact per-block lo/hi maxima under the final assignment
    src_n = new_id[src]
    dst_n = new_id[dst]
    lo_edge = src_n < SPLIT
    lo_c = np.bincount(dst_n[lo_edge], minlength=npad)
    hi_c = np.bincount(dst_n[~lo_edge], minlength=npad)
    sl = np.nonzero(order >= 0)[0]               # self loops (new ids)
    np.add.at(lo_c, sl[sl < SPLIT], 1)
    np.add.at(hi_c, sl[sl >= SPLIT], 1)

    lo_blk = lo_c.reshape(nblk, P).max(axis=1)
    hi_blk = hi_c.reshape(nblk, P).max(axis=1)
    glo = lo_blk.reshape(NCORES, bpc).max(axis=0)
    ghi = hi_blk.reshape(NCORES, bpc).max(axis=0)
    glo = glo.astype(np.int64)
    ghi = ghi.astype(np.int64)
    for i in range(bpc):
        if (glo[i] + ghi[i]) % 2:
            ghi[i] += 1
        if glo[i] + ghi[i] == 0:
            ghi[i] = 2
    return order, new_id, [int(v) for v in glo], [int(v) for v in ghi]


def host_prep(cfg, x, W1, att_src1, att_dst1, bias1, W2, att_src2,
              att_dst2, bias2, edge_index):
    n = cfg.n
    NPAD, BPC = cfg.npad, cfg.bpc
    H1, CH1, HC1 = cfg.h1, cfg.ch1, cfg.hc1
    order, new_id = cfg.order, cfg.new_id

    src = np.asarray(edge_index[0]).astype(np.int64)
    dst = np.asarray(edge_index[1]).astype(np.int64)
    loop = np.arange(n, dtype=np.int64)
    src = np.concatenate([src, loop])
    dst = np.concatenate([dst, loop])
    src_n = new_id[src]
    dst_n = new_id[dst]

    # group edges by new dst, lo-src first within each node
    lo_flag = src_n < SPLIT
    eorder = np.argsort(dst_n * 2 + (~lo_flag), kind="stable")
    src_s = src_n[eorder]
    dst_s = dst_n[eorder]
    lo_s = lo_flag[eorder]

    counts = np.bincount(dst_s, minlength=NPAD)
    lo_cnt = np.bincount(dst_s[lo_s], minlength=NPAD)
    starts = np.zeros(NPAD + 1, np.int64)
    np.cumsum(counts, out=starts[1:])
    pos_in_node = np.arange(len(src_s)) - starts[dst_s]

    SENT_LO = SPLIT - 1
    SENT_HI_REL = (NPAD - 1) - SPLIT
    blk_of = dst_s >> 7
    part_of = dst_s & 127

    SG8 = 8 * cfg.sg
    eidx_cores = [np.empty((P, SG8), np.int16) for _ in range(NCORES)]
    ibase = 0
    for bpos in range(BPC):
        glo, ghi = cfg.glo[bpos], cfg.ghi[bpos]
        G = glo + ghi
        for c in range(NCORES):
            blk = c * BPC + bpos
            grid = np.empty((P, G), np.int16)
            grid[:, :glo] = SENT_LO
            grid[:, glo:] = SENT_HI_REL
            m = blk_of == blk
            pp = part_of[m]
            sv = src_s[m]
            lv = lo_s[m]
            pn = pos_in_node[m]
            ln = lo_cnt[blk * P + pp]
            col = np.where(lv, pn, glo + (pn - ln))
            grid[pp, col] = np.where(lv, sv, sv - SPLIT).astype(np.int16)
            flat = grid.T.reshape(-1)            # k = g*128 + p
            eidx_cores[c][:, ibase:ibase + G * 8] = _wrap16(flat)
        ibase += G * 8

    # ---- parameter prep ----
    x = np.asarray(x, np.float32)
    W1 = np.asarray(W1, np.float32)
    W2 = np.asarray(W2, np.float32)
    as1 = np.asarray(att_src1, np.float32)
    ad1 = np.asarray(att_dst1, np.float32)
    as2 = np.asarray(att_src2, np.float32).reshape(-1)
    ad2 = np.asarray(att_dst2, np.float32).reshape(-1)

    A1s = np.zeros((HC1, H1), dtype=np.float32)
    A1d = np.zeros((HC1, H1), dtype=np.float32)
    hh = np.repeat(np.arange(H1), CH1)
    A1s[np.arange(HC1), hh] = as1.reshape(-1)
    A1d[np.arange(HC1), hh] = ad1.reshape(-1)
    Bs = W1 @ A1s                                   # [c_in, H1]
    # sentinel x: a_src(x_sent) == -100 per head, minimal norm
    x_sent = Bs @ np.linalg.solve(Bs.T @ Bs, -100.0 * np.ones(H1))

    xp = np.empty((NPAD, cfg.c_in), np.float32)
    real = order >= 0
    xp[real] = x[order[real]]
    xp[~real] = x_sent
    xt = np.ascontiguousarray(xp.T).astype(np.float16)

    w1aug = np.concatenate([W1, Bs, W1 @ A1d], axis=1).astype(np.float16)
    w2aug = np.concatenate([W2, (W2 @ as2)[:, None], (W2 @ ad2)[:, None]],
                           axis=1).astype(np.float16)

    colsum = W2.sum(axis=0)
    c0 = float(colsum @ (as2 + ad2))
    shifts = np.zeros((P, 2), dtype=np.float32)
    shifts[:, 0] = -2.0
    shifts[:, 1] = -c0

    b1b = np.tile(np.asarray(bias1, np.float32).reshape(1, -1),
                  (P, 1)).astype(np.float16)
    b2b = np.tile((np.asarray(bias2, np.float32).reshape(-1) - colsum
                   ).reshape(1, -1), (P, 1)).astype(np.float32)
    ident = np.eye(P, dtype=np.float16)

    # sentinel a_dst as the device computes it (fp16 inputs, fp32 matmul)
    sentadst_v = (x_sent.astype(np.float16).astype(np.float32)
                  @ w1aug[:, 264:272].astype(np.float32)).astype(np.float16)
    sentadst = np.tile(sentadst_v.reshape(1, -1), (P, 1))

    SENT_HI = (NPAD - 1) - SPLIT
    in_maps = []
    for c in range(NCORES):
        base = c * BPC * P
        pm = np.zeros((P, BPC), np.float16)
        ids = (base + np.arange(BPC)[None, :] * P +
               np.arange(P)[:, None])
        pm[:, :] = np.where(order[ids] < 0, -1000.0, 0.0)
        # own-row a_dst gather indices (lo call | hi call)
        own = base + np.arange(BPC * P, dtype=np.int64)
        if base + BPC * P <= SPLIT:
            alo, ahi = own, np.full(BPC * P, SENT_HI, np.int64)
        else:
            alo = np.full(BPC * P, SPLIT - 1, np.int64)
            ahi = own - SPLIT
        aidxv = np.concatenate(
            [_wrap16(alo.astype(np.int16)), _wrap16(ahi.astype(np.int16))],
            axis=1)
        in_maps.append({
            "xt": xt, "w1aug": w1aug, "w2aug": w2aug, "b1b": b1b,
            "b2b": b2b, "shifts": shifts, "ident": ident,
            "padmask": pm, "eidx": eidx_cores[c], "aidx": aidxv,
            "sentadst": sentadst})
    return in_maps


_prog_cache = {}
_last_results = None


def kernel(x, edge_index, edge_weight, W1, att_src1, att_dst1, bias1,
           W2, att_src2, att_dst2, bias2):
    global _last_results
    n = x.shape[0]
    # edge_weight is unused (GATConv with edge_dim=None ignores it)
    order, new_id, glo, ghi = compute_layout(n, edge_index)
    cfg = Cfg(n, glo, ghi, c_in=x.shape[1])
    cfg.order, cfg.new_id = order, new_id
    key = (cfg.n, cfg.c_in, tuple(glo), tuple(ghi))
    if key not in _prog_cache:
        _prog_cache[key] = build_program(cfg)
    nc = _prog_cache[key]

    in_maps = host_prep(cfg, x, W1, att_src1, att_dst1, bias1, W2,
                        att_src2, att_dst2, bias2, edge_index)
    res = run_bass_kernel_spmd(nc, in_maps, list(range(cfg.ncores)))
    _last_results = res
    full = np.concatenate([res.results[c]["out"]
                           for c in range(cfg.ncores)], axis=0)
    out = np.zeros((n, cfg.c2), np.float32)
    real = order >= 0
    out[order[real]] = full[real]
    return np.ascontiguousarray(out)


# revision 36
# speedup vs baseline: 1.1344x; 1.0817x over previous
"""GAT (2-layer, PyG-style) Trainium2 Bass kernel, 8-core SPMD. v4.

Strategy (dst-per-partition, reduction-based aggregation):
- Host renumbers nodes: sort by (degree desc, lo-count), deal rank-blocks
  of 128 to (core, position) so all 8 cores' blocks at the same position
  have near-equal max degree.  Core c owns contiguous new-ids
  [c*BPC*128, (c+1)*BPC*128).  Partition p of block b IS dst node; its
  edges lie along the free dim as [lo-src | pad | hi-src | pad], padded
  to per-position (Glo, Ghi) maxima over cores (the compiled program is
  shared by all cores).
- a_dst is a per-partition scalar (block-local SBUF table from phase 0)
  => no a_dst gather, no one-hot scatter matmuls.  Segment softmax +
  scatter-add become per-partition row ops + a free-dim tree reduction.
- h/a_src rows are fetched with InstDMAGatherAnt (int16 idx) from node
  tables split at row 25088 so both halves' indices fit int16. Pad slots
  gather a sentinel row whose a_src is -100 => exp()==0 in fp16, so
  padding contributes exactly zero to message and denominator sums.
- Logits are exp-shifted by -2 (cancels in softmax; keeps fp16 finite).
  ELU's "-1" is dropped (eluplus = relu(x)+exp(min(x,0))) and corrected
  at the end: out -= colsum(W2); the induced constant layer-2 logit
  shift c0 is subtracted pre-leaky_relu.  Sentinel layer-2 a_src is
  forced to -1000 via a padmask input baked in before the AllGather.
- One AllGather of the 128-col fp16 layer-2 node table is the only
  collective.
"""

import numpy as np

import concourse.bacc as bacc
import concourse.mybir as mybir
import concourse.tile as tile
from concourse.bass_utils import run_bass_kernel_spmd
from bass_rust import add_dep_helper


def _dep(a, b, reason):
    ia = a.ins if hasattr(a, "ins") else a
    ib = b.ins if hasattr(b, "ins") else b
    add_dep_helper(ia, ib, reason=reason)


P = 128
NCORES = 8
EPS = 1e-16
NEG_SLOPE = 0.2
SPLIT = 25088               # node-table split so gather idx fits int16
F32 = mybir.dt.float32
F16 = mybir.dt.float16
I32 = mybir.dt.int32
I16 = mybir.dt.int16
AF = mybir.ActivationFunctionType
ALU = mybir.AluOpType

MAXCOLS = 8                 # <=8 cols (1024 idx) per gather call
GCAP = 24                   # max grid columns per processing segment


class Cfg:
    def __init__(self, n_nodes, glo, ghi, c_in=128, h1=8, ch1=32, c2=64,
                 ncores=NCORES):
        self.n = n_nodes
        self.c_in = c_in
        self.h1 = h1
        self.ch1 = ch1
        self.hc1 = h1 * ch1          # 256
        self.c2 = c2
        self.ncores = ncores
        self.bpc = -(-n_nodes // (P * ncores))      # 49
        self.npad = ncores * self.bpc * P
        self.nblk = ncores * self.bpc
        self.t1w = 384               # [h 256 | asrc 8 | adst 8 | junk]
        self.t2w = 128               # [h2 64 | asrc2 1 | adst2 1 | junk]
        self.glo = list(glo)         # per block position, len bpc
        self.ghi = list(ghi)
        self.g = [a + b for a, b in zip(self.glo, self.ghi)]
        self.sg = sum(self.g)
        self.gmax = max(self.g)


_GQ = [0]


def build_program(cfg):
    nc = bacc.Bacc(None, num_devices=cfg.ncores, num_swdge_queues=4,
                   dynamic_dma_scratch_size=16384)
    HC1, H1, CH1, C2 = cfg.hc1, cfg.h1, cfg.ch1, cfg.c2
    T1W, T2W, BPC = cfg.t1w, cfg.t2w, cfg.bpc
    NBLK, NPAD = cfg.nblk, cfg.npad
    G1 = 8
    G0 = -(-BPC // G1)              # phase-0 groups of 8 blocks

    # ---- I/O ----
    xt = nc.dram_tensor("xt", [cfg.c_in, NPAD], F16, kind="ExternalInput")
    w1aug = nc.dram_tensor("w1aug", [cfg.c_in, 272], F16, kind="ExternalInput")
    w2aug = nc.dram_tensor("w2aug", [HC1, 66], F16, kind="ExternalInput")
    b1b = nc.dram_tensor("b1b", [P, HC1], F16, kind="ExternalInput")
    b2b = nc.dram_tensor("b2b", [P, C2], F32, kind="ExternalInput")
    shifts = nc.dram_tensor("shifts", [P, 2], F32, kind="ExternalInput")
    ident = nc.dram_tensor("ident", [P, P], F16, kind="ExternalInput")
    padmask = nc.dram_tensor("padmask", [P, BPC], F16, kind="ExternalInput")
    eidx = nc.dram_tensor("eidx", [P, 8 * cfg.sg], I16, kind="ExternalInput")
    aidx = nc.dram_tensor("aidx", [P, 2 * BPC * 8], I16, kind="ExternalInput")
    sentadst = nc.dram_tensor("sentadst", [P, H1], F16, kind="ExternalInput")
    out = nc.dram_tensor("out", [BPC * P, C2], F32, kind="ExternalOutput")

    # ---- internal DRAM ----
    t1 = nc.dram_tensor("t1", [NPAD, T1W], F16)
    cc_in = nc.dram_tensor("cc_in", [BPC * P, T2W], F16)
    t2 = nc.dram_tensor("t2", [NPAD, T2W], F16, addr_space="Shared")

    groups = [list(range(cfg.ncores))]
    GM = cfg.gmax

    with tile.TileContext(nc) as tc:
        with (
            tc.tile_pool(name="const", bufs=1) as cpool,
            tc.tile_pool(name="p1", bufs=2) as p1pool,
            tc.tile_pool(name="gat", bufs=5) as gpool,
            tc.tile_pool(name="blk", bufs=3) as bpool,
            tc.tile_pool(name="fin", bufs=4) as opool,
            tc.tile_pool(name="ps", bufs=2, space="PSUM") as ps,
        ):
            # ---------------- constants ----------------
            ident_s = cpool.tile([P, P], F16)
            nc.sync.dma_start(out=ident_s[:], in_=ident[:])
            w1aug_s = cpool.tile([P, 272], F16)
            nc.sync.dma_start(out=w1aug_s[:], in_=w1aug[:])
            w2aug_s = []
            for j in range(HC1 // P):
                wg = cpool.tile([P, 66], F16, tag=f"w2aug{j}")
                nc.sync.dma_start(out=wg[:], in_=w2aug[j * P:(j + 1) * P, :])
                w2aug_s.append(wg)
            b1b_s = cpool.tile([P, HC1], F16)
            nc.sync.dma_start(out=b1b_s[:], in_=b1b[:])
            b2b_s = cpool.tile([P, C2], F32)
            nc.sync.dma_start(out=b2b_s[:], in_=b2b[:])
            shifts_s = cpool.tile([P, 2], F32)
            nc.sync.dma_start(out=shifts_s[:], in_=shifts[:])
            padmask_s = cpool.tile([P, BPC], F16)
            nc.sync.dma_start(out=padmask_s[:], in_=padmask[:])
            aidx_s = cpool.tile([P, 2 * BPC * 8], I16)
            nc.sync.dma_start(out=aidx_s[:], in_=aidx[:])
            sentadst_s = cpool.tile([P, H1], F16)
            nc.sync.dma_start(out=sentadst_s[:], in_=sentadst[:])
            eidx_s = cpool.tile([P, 8 * cfg.sg], I16)
            nc.sync.dma_start(out=eidx_s[:], in_=eidx[:])
            # persistent per-core tables
            A1 = cpool.tile([P, BPC * H1], F16)       # own-node a_dst
            A2 = cpool.tile([P, BPC], F16)            # own-node a_dst2

            # ------------- phase 1: full node table (replicated) ---------
            t1_writes = []
            for grp in range(NBLK // G1):
                B0 = grp * G1
                xg = p1pool.tile([P, G1 * P], F16, tag="xg")
                nc.sync.dma_start(out=xg[:], in_=xt[:, B0 * P:(B0 + G1) * P])
                rows = p1pool.tile([P, G1 * 272], F16, tag="rows")
                for j in range(G1):
                    ph1 = ps.tile([P, 272], F32, space="PSUM", tag="acc")
                    nc.tensor.matmul(out=ph1[:], lhsT=xg[:, j * P:(j + 1) * P],
                                     rhs=w1aug_s[:], start=True, stop=True)
                    dst = rows[:, j * 272:(j + 1) * 272]
                    if j % 2 == 0:
                        nc.scalar.copy(out=dst, in_=ph1[:])
                    else:
                        nc.vector.tensor_scalar_add(out=dst, in0=ph1[:],
                                                    scalar1=0.0)
                t1_writes.append(nc.sync.dma_start(
                    out=t1[B0 * P:(B0 + G1) * P, 0:272].rearrange(
                        "(j p) c -> p j c", j=G1),
                    in_=rows[:].rearrange("p (j c) -> p j c", j=G1)))

            j1tile = cpool.tile([1, 1], F32, tag="j1")
            j1 = nc.gpsimd.memset(j1tile[:], 0.0)
            for w in t1_writes:
                _dep(j1, w, "layer1 gathers wait for full node table")

            # ------------- phase 0b: own-node a_dst from t1 --------------
            # Per core all own rows sit in one table half; the other half's
            # call gathers only the sentinel row (idx data decides), and the
            # host-supplied sentinel a_dst is subtracted back out.
            MA = 8                               # block-cols per piece
            for k0 in range(0, BPC, MA):
                nk = min(MA, BPC - k0)
                bufs = []
                for half in range(2):
                    buf = p1pool.tile([P, MA * P], F16, tag=f"a1g{half}")
                    tslc = t1[SPLIT:NPAD, 256:384] if half \
                        else t1[0:SPLIT, 256:384]
                    q = _GQ[0] % 4
                    _GQ[0] += 1
                    g = nc.gpsimd.dma_gather(
                        out_ap=buf[:, 0:nk * P]
                            .rearrange("p (m w) -> p m w", m=nk),
                        in_ap=tslc,
                        idxs_ap=aidx_s[:, half * BPC * 8 + k0 * 8:
                                       half * BPC * 8 + (k0 + nk) * 8],
                        num_idxs=nk * P, num_idxs_reg=nk * P,
                        elem_size=P, elem_step=T1W, queue_num=q)
                    _dep(g, j1, "a_dst gather after t1")
                    bufs.append(buf)
                # adst cols are 8:16 of the gathered 128-col window
                asl = A1[:, k0 * H1:(k0 + nk) * H1]
                nc.vector.tensor_tensor(
                    out=asl.rearrange("p (b h) -> p b h", b=nk),
                    in0=bufs[0][:, 0:nk * P]
                        .rearrange("p (b w) -> p b w", b=nk)[:, :, 8:16],
                    in1=bufs[1][:, 0:nk * P]
                        .rearrange("p (b w) -> p b w", b=nk)[:, :, 8:16],
                    op=ALU.add)
                nc.vector.tensor_tensor(
                    out=asl.rearrange("p (b h) -> p b h", b=nk),
                    in0=asl.rearrange("p (b h) -> p b h", b=nk),
                    in1=sentadst_s[:].unsqueeze(1).to_broadcast([P, nk, H1]),
                    op=ALU.subtract)

            def tree_reduce(width, G, bufA, bufB):
                """Sum bufA[:, :G*width] over the G groups; ping-pongs
                between bufA and bufB. Returns the buffer whose cols
                [0:width] hold the sums."""
                src, dstb, n = bufA, bufB, G
                while n > 1:
                    h = n // 2
                    odd = n - 2 * h
                    nc.vector.tensor_tensor(
                        out=dstb[:, 0:h * width],
                        in0=src[:, 0:h * width],
                        in1=src[:, h * width:2 * h * width],
                        op=ALU.add)
                    if odd:
                        nc.vector.tensor_tensor(
                            out=dstb[:, 0:width],
                            in0=dstb[:, 0:width],
                            in1=src[:, 2 * h * width:(2 * h + 1) * width],
                            op=ALU.add)
                    src, dstb = dstb, src
                    n = h
                return src

            # ------------- phase 2: layer-1 per-dst aggregation ----------
            # Blocks are processed in column segments of <= GCAP so gather
            # tiles are small enough for a deep (bufs=4) pipeline; segment
            # partial sums accumulate into a per-block accumulator.
            W264 = HC1 + H1

            def segments(G):
                k = -(-G // GCAP)
                bounds = [2 * round(i * G / k / 2) for i in range(k + 1)]
                bounds[-1] = G
                return list(zip(bounds[:-1], bounds[1:]))

            def gather_seg(b, c0, c1, grow, tab, elem, dep, why):
                """Gather grid columns [c0, c1) of block b into grow."""
                glo = cfg.glo[b]
                ibase = 8 * sum(cfg.g[:b])
                nseg = c1 - c0
                ranges = []
                if c0 < glo:
                    ranges.append((c0, min(c1, glo), 0))
                if c1 > glo:
                    ranges.append((max(c0, glo), c1, 1))
                for (r0_, r1_, hi) in ranges:
                    tslc = tab[SPLIT:NPAD, 0:elem] if hi \
                        else tab[0:SPLIT, 0:elem]
                    for k0 in range(r0_, r1_, MAXCOLS):
                        nk = min(MAXCOLS, r1_ - k0)
                        q = _GQ[0] % 4
                        _GQ[0] += 1
                        g = nc.gpsimd.dma_gather(
                            out_ap=grow[:, (k0 - c0) * elem:
                                        (k0 - c0 + nk) * elem]
                                .rearrange("p (m w) -> p m w", m=nk),
                            in_ap=tslc,
                            idxs_ap=eidx_s[:, ibase + k0 * 8:
                                           ibase + (k0 + nk) * 8],
                            num_idxs=nk * 128, num_idxs_reg=nk * 128,
                            elem_size=elem, queue_num=q)
                        _dep(g, dep, why)

            cc_writes = []
            for b in range(BPC):
                G = cfg.g[b]
                r0 = b * P
                segs = segments(G)
                acc = opool.tile([P, W264], F16, tag="acc1")
                for si, (c0, c1) in enumerate(segs):
                    ncol = c1 - c0
                    grow = gpool.tile([P, GCAP * T1W], F16, tag="grow")
                    gather_seg(b, c0, c1, grow, t1, T1W, j1, "l1 gather")
                    grv = grow[:, :ncol * T1W].rearrange(
                        "p (g c) -> p g c", g=ncol)

                    av = bpool.tile([P, GCAP * H1], F16, tag="av")
                    nc.vector.tensor_tensor(
                        out=av[:, :ncol * H1].rearrange(
                            "p (g h) -> p g h", g=ncol),
                        in0=grv[:, :, 256:264],
                        in1=A1[:, b * H1:(b + 1) * H1].unsqueeze(1)
                            .to_broadcast([P, ncol, H1]),
                        op=ALU.add)
                    lk = bpool.tile([P, GCAP * H1], F16, tag="lk")
                    nc.vector.scalar_tensor_tensor(
                        out=lk[:, :ncol * H1], in0=av[:, :ncol * H1],
                        scalar=NEG_SLOPE, in1=av[:, :ncol * H1],
                        op0=ALU.mult, op1=ALU.max)

                    wm = bpool.tile([P, GCAP * W264], F16, tag="wm")
                    wmv = wm[:, :ncol * W264].rearrange(
                        "p (g c) -> p g c", g=ncol)
                    # exp, broadcast 8 heads -> 256 lanes, on the scalar
                    # engine, directly into wm's message columns
                    nc.scalar.activation(
                        out=wmv[:, :, 0:HC1].rearrange(
                            "p g (h c) -> p g h c", h=H1),
                        in_=lk[:, :ncol * H1].rearrange(
                            "p (g h) -> p g h", g=ncol)
                            .unsqueeze(3).to_broadcast([P, ncol, H1, CH1]),
                        func=AF.Exp, bias=shifts_s[:, 0:1])
                    # denominator columns: one e per head (stride-32 picks)
                    nc.vector.tensor_copy(
                        out=wmv[:, :, HC1:W264],
                        in_=wmv[:, :, 0:HC1].rearrange(
                            "p g (h c) -> p g h c", h=H1)[:, :, :, 0])
                    nc.vector.tensor_tensor(
                        out=wmv[:, :, 0:HC1],
                        in0=grv[:, :, 0:HC1],
                        in1=wmv[:, :, 0:HC1],
                        op=ALU.mult)

                    redt = bpool.tile([P, (GCAP // 2 + 1) * W264], F16,
                                      tag="redt")
                    red = tree_reduce(W264, ncol, wm, redt)
                    if si == 0:
                        nc.vector.tensor_copy(out=acc[:],
                                              in_=red[:, 0:W264])
                    else:
                        nc.vector.tensor_tensor(
                            out=acc[:], in0=acc[:], in1=red[:, 0:W264],
                            op=ALU.add)

                msum = acc[:, 0:HC1]
                dsum = acc[:, HC1:W264]
                denf = opool.tile([P, H1], F32, tag="denf")
                nc.vector.tensor_scalar_add(out=denf[:], in0=dsum,
                                            scalar1=EPS)
                rec = opool.tile([P, H1], F32, tag="rec")
                nc.vector.reciprocal(out=rec[:], in_=denf[:])
                recb = opool.tile([P, HC1], F16, tag="recb")
                nc.vector.tensor_scalar(
                    out=recb[:].rearrange("p (h c) -> p h c", h=H1),
                    in0=rec[:].unsqueeze(2).to_broadcast([P, H1, CH1]),
                    scalar1=60000.0, scalar2=None, op0=ALU.min)
                o1b = opool.tile([P, HC1], F16, tag="o1b")
                nc.vector.tensor_tensor(out=o1b[:], in0=msum, in1=recb[:],
                                        op=ALU.mult)
                nc.vector.tensor_tensor(out=o1b[:], in0=o1b[:], in1=b1b_s[:],
                                        op=ALU.add)
                xn = opool.tile([P, HC1], F16, tag="recb")
                nc.vector.tensor_scalar_min(out=xn[:], in0=o1b[:], scalar1=0.0)
                en = opool.tile([P, HC1], F16, tag="recb")
                nc.scalar.activation(out=en[:], in_=xn[:], func=AF.Exp)
                helu = opool.tile([P, HC1], F16, tag="o1b")
                nc.vector.scalar_tensor_tensor(
                    out=helu[:], in0=o1b[:], scalar=0.0, in1=en[:],
                    op0=ALU.max, op1=ALU.add)

                ph2 = ps.tile([P, 66], F32, space="PSUM", tag="ph2")
                for j in range(HC1 // P):
                    pT = ps.tile([P, P], F16, space="PSUM", tag="pT")
                    nc.tensor.transpose(out=pT[:],
                                        in_=helu[:, j * P:(j + 1) * P],
                                        identity=ident_s[:])
                    hT = opool.tile([P, P], F16, tag="hT")
                    nc.scalar.copy(out=hT[:], in_=pT[:])
                    nc.tensor.matmul(out=ph2[:], lhsT=hT[:], rhs=w2aug_s[j][:],
                                     start=(j == 0), stop=(j == HC1 // P - 1))
                h2row = opool.tile([P, 66], F16, tag="h2row")
                nc.scalar.copy(out=h2row[:, 0:66], in_=ph2[:])
                # force sentinel/pad-node layer-2 a_src very negative
                nc.vector.tensor_tensor(
                    out=h2row[:, 64:65], in0=h2row[:, 64:65],
                    in1=padmask_s[:, b:b + 1], op=ALU.add)
                nc.scalar.copy(out=A2[:, b:b + 1], in_=h2row[:, 65:66])
                cc_writes.append(nc.sync.dma_start(
                    out=cc_in[r0:r0 + P, 0:66], in_=h2row[:]))

            # ------------- phase 3: share layer-2 node table -------------
            cc = nc.gpsimd.collective_compute(
                "AllGather", ALU.bypass, replica_groups=groups,
                ins=[cc_in[:]], outs=[t2[:]])
            for w in cc_writes:
                _dep(cc, w, "allgather after cc writes")
            j2tile = cpool.tile([1, 1], F32, tag="j2")
            j2 = nc.gpsimd.memset(j2tile[:], 0.0)
            _dep(j2, cc, "layer2 gathers after allgather")

            # ------------- phase 4: layer-2 per-dst aggregation ----------
            for b in range(BPC):
                G = cfg.g[b]
                r0 = b * P
                segs = segments(G)
                acc2 = opool.tile([P, C2], F16, tag="acc2")
                acc2d = opool.tile([P, 2], F32, tag="acc2d")
                for si, (c0, c1) in enumerate(segs):
                    ncol = c1 - c0
                    grow2 = gpool.tile([P, GCAP * T1W], F16, tag="grow")
                    gather_seg(b, c0, c1, grow2, t2, T2W, j2, "l2 gather")
                    grv2 = grow2[:, :ncol * T2W].rearrange(
                        "p (g c) -> p g c", g=ncol)

                    av2 = bpool.tile([P, GCAP], F16, tag="av2")
                    nc.vector.scalar_tensor_tensor(
                        out=av2[:, :ncol],
                        in0=grv2[:, :, 64:65].rearrange("p g o -> p (g o)"),
                        scalar=shifts_s[:, 1:2],
                        in1=A2[:, b:b + 1].to_broadcast([P, ncol]),
                        op0=ALU.add, op1=ALU.add)
                    lk2 = bpool.tile([P, GCAP], F16, tag="lk2")
                    nc.vector.scalar_tensor_tensor(
                        out=lk2[:, :ncol], in0=av2[:, :ncol], scalar=NEG_SLOPE,
                        in1=av2[:, :ncol], op0=ALU.mult, op1=ALU.max)
                    e2 = bpool.tile([P, GCAP], F16, tag="e2")
                    nc.scalar.activation(out=e2[:, :ncol], in_=lk2[:, :ncol],
                                         func=AF.Exp, bias=shifts_s[:, 0:1])

                    wm2 = bpool.tile([P, GCAP * C2], F16, tag="wm")
                    wm2v = wm2[:, :ncol * C2].rearrange(
                        "p (g c) -> p g c", g=ncol)
                    nc.vector.tensor_tensor(
                        out=wm2v[:],
                        in0=grv2[:, :, 0:C2],
                        in1=e2[:, :ncol].unsqueeze(2)
                            .to_broadcast([P, ncol, C2]),
                        op=ALU.mult)

                    redt2 = bpool.tile([P, (GCAP // 2 + 1) * C2], F16,
                                       tag="redt")
                    red2 = tree_reduce(C2, ncol, wm2, redt2)
                    dpart = opool.tile([P, 2], F32, tag="dpart")
                    nc.vector.tensor_reduce(
                        out=dpart[:, 0:1], in_=e2[:, :ncol],
                        axis=mybir.AxisListType.X, op=ALU.add)
                    if si == 0:
                        nc.vector.tensor_copy(out=acc2[:], in_=red2[:, 0:C2])
                        nc.vector.tensor_copy(out=acc2d[:, 0:1],
                                              in_=dpart[:, 0:1])
                    else:
                        nc.vector.tensor_tensor(
                            out=acc2[:], in0=acc2[:], in1=red2[:, 0:C2],
                            op=ALU.add)
                        nc.vector.tensor_tensor(
                            out=acc2d[:, 0:1], in0=acc2d[:, 0:1],
                            in1=dpart[:, 0:1], op=ALU.add)

                den2f = opool.tile([P, 1], F32, tag="den2f")
                nc.vector.tensor_scalar_add(out=den2f[:],
                                            in0=acc2d[:, 0:1],
                                            scalar1=EPS)
                rec2 = opool.tile([P, 1], F32, tag="rec2")
                nc.vector.reciprocal(out=rec2[:], in_=den2f[:])
                o2 = opool.tile([P, C2], F32, tag="o2")
                nc.vector.scalar_tensor_tensor(
                    out=o2[:], in0=acc2[:], scalar=rec2[:, 0:1],
                    in1=b2b_s[:], op0=ALU.mult, op1=ALU.add)
                nc.sync.dma_start(out=out[r0:r0 + P, :], in_=o2[:])

    nc.compile()
    return nc


def _wrap16(idx):
    """Pack int16 idx array (len multiple of 128) into wrapped-16 layout
    [128, n//16]: element k at (k%16, k//16), replicated to rows 16..127."""
    n = len(idx)
    a = np.asarray(idx, np.int16).reshape(n // 16, 16).T  # [16, n//16]
    return np.tile(a, (8, 1))


def _deal_half(order, ranked_nodes, id0, half_cores, bpc):
    """Deal deg-sorted nodes of one class into its 4 cores' id range:
    consecutive 128-node blocks go to (core stripe, position) pairs."""
    nh = len(ranked_nodes)
    r = np.arange(nh)
    blk = r >> 7
    ids = (id0 + ((blk % half_cores) * bpc + blk // half_cores) * P
           + (r & 127))
    order[ids] = ranked_nodes


def compute_layout(n, edge_index):
    """Node permutation + per-position (Glo, Ghi) column counts."""
    bpc = -(-n // (P * NCORES))
    npad = NCORES * bpc * P
    nblk = NCORES * bpc
    half = npad // 2                 # == SPLIT for the 8-core layout
    assert half == SPLIT

    src = np.asarray(edge_index[0]).astype(np.int64)
    dst = np.asarray(edge_index[1]).astype(np.int64)
    deg = np.bincount(dst, minlength=n) + 1      # + self loop

    # fix each node's lo/hi CLASS up front (alternating deg-rank blocks
    # so both halves get identical degree profiles); classes never move,
    # so per-node lo-counts are exact, not a fixed-point guess.
    rank_of = np.argsort(-deg, kind="stable")    # rank -> orig node
    rank_inv = np.empty(n, np.int64)
    rank_inv[rank_of] = np.arange(n)
    is_lo = ((rank_inv >> 7) % NCORES) < (NCORES // 2)   # per orig node
    # keep >=1 pad id in each half for its sentinel row
    for flip_from, flag in ((is_lo, True), (~is_lo, False)):
        excess = int(flip_from.sum()) - (half - 1)
        if excess > 0:
            cand = np.nonzero(flip_from)[0]
            worst = cand[np.argsort(rank_inv[cand])[-excess:]]
            is_lo[worst] = not flag

    lo_cnt = np.bincount(dst[is_lo[src]], minlength=n)
    lo_cnt += is_lo                               # self loop
    hi_cnt = deg - lo_cnt
    key = lo_cnt.astype(np.int64) * (4 * npad) + hi_cnt
    krank = np.argsort(key, kind="stable")

    order = np.full(npad, -1, np.int64)           # new id -> orig node
    lo_nodes = krank[is_lo[krank]]
    hi_nodes = krank[~is_lo[krank]]
    assert len(lo_nodes) <= half and len(hi_nodes) <= half
    _deal_half(order, lo_nodes, 0, NCORES // 2, bpc)
    _deal_half(order, hi_nodes, half, NCORES // 2, bpc)

    # force sentinel ids (last row of each table half) to be pads
    for sent, lim0, lim1 in ((SPLIT - 1, 0, half), (npad - 1, half, npad)):
        if order[sent] >= 0:
            padq = np.nonzero(order[lim0:lim1] < 0)[0]
            assert len(padq), "no pad id available in half"
            q = lim0 + padq[-1]
            order[q] = order[sent]
            order[sent] = -1
    new_id = np.full(n, -1, np.int64)
    real = np.nonzero(order >= 0)[0]
    new_id[order[real]] = real
    # classes preserved by construction
    assert (new_id[lo_nodes] < SPLIT).all()
    assert (new_id[hi_nodes] >= SPLIT).all()

    # exact per-block lo/hi maxima under the final assignment
    src_n = new_id[src]
    dst_n = new_id[dst]
    lo_edge = src_n < SPLIT
    lo_c = np.bincount(dst_n[lo_edge], minlength=npad)
    hi_c = np.bincount(dst_n[~lo_edge], minlength=npad)
    sl = np.nonzero(order >= 0)[0]               # self loops (new ids)
    np.add.at(lo_c, sl[sl < SPLIT], 1)
    np.add.at(hi_c, sl[sl >= SPLIT], 1)

    lo_blk = lo_c.reshape(nblk, P).max(axis=1)
    hi_blk = hi_c.reshape(nblk, P).max(axis=1)
    glo = lo_blk.reshape(NCORES, bpc).max(axis=0)
    ghi = hi_blk.reshape(NCORES, bpc).max(axis=0)
    glo = glo.astype(np.int64)
    ghi = ghi.astype(np.int64)
    for i in range(bpc):
        if (glo[i] + ghi[i]) % 2:
            ghi[i] += 1
        if glo[i] + ghi[i] == 0:
            ghi[i] = 2
    return order, new_id, [int(v) for v in glo], [int(v) for v in ghi]


def host_prep(cfg, x, W1, att_src1, att_dst1, bias1, W2, att_src2,
              att_dst2, bias2, edge_index):
    n = cfg.n
    NPAD, BPC = cfg.npad, cfg.bpc
    H1, CH1, HC1 = cfg.h1, cfg.ch1, cfg.hc1
    order, new_id = cfg.order, cfg.new_id

    src = np.asarray(edge_index[0]).astype(np.int64)
    dst = np.asarray(edge_index[1]).astype(np.int64)
    loop = np.arange(n, dtype=np.int64)
    src = np.concatenate([src, loop])
    dst = np.concatenate([dst, loop])
    src_n = new_id[src]
    dst_n = new_id[dst]

    # group edges by new dst, lo-src first within each node
    lo_flag = src_n < SPLIT
    eorder = np.argsort(dst_n * 2 + (~lo_flag), kind="stable")
    src_s = src_n[eorder]
    dst_s = dst_n[eorder]
    lo_s = lo_flag[eorder]

    counts = np.bincount(dst_s, minlength=NPAD)
    lo_cnt = np.bincount(dst_s[lo_s], minlength=NPAD)
    starts = np.zeros(NPAD + 1, np.int64)
    np.cumsum(counts, out=starts[1:])
    pos_in_node = np.arange(len(src_s)) - starts[dst_s]

    SENT_LO = SPLIT - 1
    SENT_HI_REL = (NPAD - 1) - SPLIT
    blk_of = dst_s >> 7
    part_of = dst_s & 127

    SG8 = 8 * cfg.sg
    eidx_cores = [np.empty((P, SG8), np.int16) for _ in range(NCORES)]
    ibase = 0
    for bpos in range(BPC):
        glo, ghi = cfg.glo[bpos], cfg.ghi[bpos]
        G = glo + ghi
        for c in range(NCORES):
            blk = c * BPC + bpos
            grid = np.empty((P, G), np.int16)
            grid[:, :glo] = SENT_LO
            grid[:, glo:] = SENT_HI_REL
            m = blk_of == blk
            pp = part_of[m]
            sv = src_s[m]
            lv = lo_s[m]
            pn = pos_in_node[m]
            ln = lo_cnt[blk * P + pp]
            col = np.where(lv, pn, glo + (pn - ln))
            grid[pp, col] = np.where(lv, sv, sv - SPLIT).astype(np.int16)
            flat = grid.T.reshape(-1)            # k = g*128 + p
            eidx_cores[c][:, ibase:ibase + G * 8] = _wrap16(flat)
        ibase += G * 8

    # ---- parameter prep ----
    x = np.asarray(x, np.float32)
    W1 = np.asarray(W1, np.float32)
    W2 = np.asarray(W2, np.float32)
    as1 = np.asarray(att_src1, np.float32)
    ad1 = np.asarray(att_dst1, np.float32)
    as2 = np.asarray(att_src2, np.float32).reshape(-1)
    ad2 = np.asarray(att_dst2, np.float32).reshape(-1)

    A1s = np.zeros((HC1, H1), dtype=np.float32)
    A1d = np.zeros((HC1, H1), dtype=np.float32)
    hh = np.repeat(np.arange(H1), CH1)
    A1s[np.arange(HC1), hh] = as1.reshape(-1)
    A1d[np.arange(HC1), hh] = ad1.reshape(-1)
    Bs = W1 @ A1s                                   # [c_in, H1]
    # sentinel x: a_src(x_sent) == -100 per head, minimal norm
    x_sent = Bs @ np.linalg.solve(Bs.T @ Bs, -100.0 * np.ones(H1))

    xp = np.empty((NPAD, cfg.c_in), np.float32)
    real = order >= 0
    xp[real] = x[order[real]]
    xp[~real] = x_sent
    xt = np.ascontiguousarray(xp.T).astype(np.float16)

    w1aug = np.concatenate([W1, Bs, W1 @ A1d], axis=1).astype(np.float16)
    w2aug = np.concatenate([W2, (W2 @ as2)[:, None], (W2 @ ad2)[:, None]],
                           axis=1).astype(np.float16)

    colsum = W2.sum(axis=0)
    c0 = float(colsum @ (as2 + ad2))
    shifts = np.zeros((P, 2), dtype=np.float32)
    shifts[:, 0] = -2.0
    shifts[:, 1] = -c0

    b1b = np.tile(np.asarray(bias1, np.float32).reshape(1, -1),
                  (P, 1)).astype(np.float16)
    b2b = np.tile((np.asarray(bias2, np.float32).reshape(-1) - colsum
                   ).reshape(1, -1), (P, 1)).astype(np.float32)
    ident = np.eye(P, dtype=np.float16)

    # sentinel a_dst as the device computes it (fp16 inputs, fp32 matmul)
    sentadst_v = (x_sent.astype(np.float16).astype(np.float32)
                  @ w1aug[:, 264:272].astype(np.float32)).astype(np.float16)
    sentadst = np.tile(sentadst_v.reshape(1, -1), (P, 1))

    SENT_HI = (NPAD - 1) - SPLIT
    in_maps = []
    for c in range(NCORES):
        base = c * BPC * P
        pm = np.zeros((P, BPC), np.float16)
        ids = (base + np.arange(BPC)[None, :] * P +
               np.arange(P)[:, None])
        pm[:, :] = np.where(order[ids] < 0, -1000.0, 0.0)
        # own-row a_dst gather indices (lo call | hi call)
        own = base + np.arange(BPC * P, dtype=np.int64)
        if base + BPC * P <= SPLIT:
            alo, ahi = own, np.full(BPC * P, SENT_HI, np.int64)
        else:
            alo = np.full(BPC * P, SPLIT - 1, np.int64)
            ahi = own - SPLIT
        aidxv = np.concatenate(
            [_wrap16(alo.astype(np.int16)), _wrap16(ahi.astype(np.int16))],
            axis=1)
        in_maps.append({
            "xt": xt, "w1aug": w1aug, "w2aug": w2aug, "b1b": b1b,
            "b2b": b2b, "shifts": shifts, "ident": ident,
            "padmask": pm, "eidx": eidx_cores[c], "aidx": aidxv,
            "sentadst": sentadst})
    return in_maps


_prog_cache = {}
_last_results = None


def kernel(x, edge_index, edge_weight, W1, att_src1, att_dst1, bias1,
           W2, att_src2, att_dst2, bias2):
    global _last_results
    n = x.shape[0]
    # edge_weight is unused (GATConv with edge_dim=None ignores it)
    order, new_id, glo, ghi = compute_layout(n, edge_index)
    cfg = Cfg(n, glo, ghi, c_in=x.shape[1])
    cfg.order, cfg.new_id = order, new_id
    key = (cfg.n, cfg.c_in, tuple(glo), tuple(ghi))
    if key not in _prog_cache:
        _prog_cache[key] = build_program(cfg)
    nc = _prog_cache[key]

    in_maps = host_prep(cfg, x, W1, att_src1, att_dst1, bias1, W2,
                        att_src2, att_dst2, bias2, edge_index)
    res = run_bass_kernel_spmd(nc, in_maps, list(range(cfg.ncores)))
    _last_results = res
    full = np.concatenate([res.results[c]["out"]
                           for c in range(cfg.ncores)], axis=0)
    out = np.zeros((n, cfg.c2), np.float32)
    real = order >= 0
    out[order[real]] = full[real]
    return np.ascontiguousarray(out)


# revision 39
# speedup vs baseline: 1.1376x; 1.0028x over previous
"""GAT (2-layer, PyG-style) Trainium2 Bass kernel, 8-core SPMD. v4.

Strategy (dst-per-partition, reduction-based aggregation):
- Host renumbers nodes: sort by (degree desc, lo-count), deal rank-blocks
  of 128 to (core, position) so all 8 cores' blocks at the same position
  have near-equal max degree.  Core c owns contiguous new-ids
  [c*BPC*128, (c+1)*BPC*128).  Partition p of block b IS dst node; its
  edges lie along the free dim as [lo-src | pad | hi-src | pad], padded
  to per-position (Glo, Ghi) maxima over cores (the compiled program is
  shared by all cores).
- a_dst is a per-partition scalar (block-local SBUF table from phase 0)
  => no a_dst gather, no one-hot scatter matmuls.  Segment softmax +
  scatter-add become per-partition row ops + a free-dim tree reduction.
- h/a_src rows are fetched with InstDMAGatherAnt (int16 idx) from node
  tables split at row 25088 so both halves' indices fit int16. Pad slots
  gather a sentinel row whose a_src is -100 => exp()==0 in fp16, so
  padding contributes exactly zero to message and denominator sums.
- Logits are exp-shifted by -2 (cancels in softmax; keeps fp16 finite).
  ELU's "-1" is dropped (eluplus = relu(x)+exp(min(x,0))) and corrected
  at the end: out -= colsum(W2); the induced constant layer-2 logit
  shift c0 is subtracted pre-leaky_relu.  Sentinel layer-2 a_src is
  forced to -1000 via a padmask input baked in before the AllGather.
- One AllGather of the 128-col fp16 layer-2 node table is the only
  collective.
"""

import numpy as np

import concourse.bacc as bacc
import concourse.mybir as mybir
import concourse.tile as tile
from concourse.bass_utils import run_bass_kernel_spmd
from bass_rust import add_dep_helper


def _dep(a, b, reason):
    ia = a.ins if hasattr(a, "ins") else a
    ib = b.ins if hasattr(b, "ins") else b
    add_dep_helper(ia, ib, reason=reason)


P = 128
NCORES = 8
EPS = 1e-16
NEG_SLOPE = 0.2
SPLIT = 25088               # node-table split so gather idx fits int16
F32 = mybir.dt.float32
F16 = mybir.dt.float16
I32 = mybir.dt.int32
I16 = mybir.dt.int16
AF = mybir.ActivationFunctionType
ALU = mybir.AluOpType

MAXCOLS = 8                 # <=8 cols (1024 idx) per gather call
GCAP = 24                   # max grid columns per processing segment


class Cfg:
    def __init__(self, n_nodes, glo, ghi, c_in=128, h1=8, ch1=32, c2=64,
                 ncores=NCORES):
        self.n = n_nodes
        self.c_in = c_in
        self.h1 = h1
        self.ch1 = ch1
        self.hc1 = h1 * ch1          # 256
        self.c2 = c2
        self.ncores = ncores
        self.bpc = -(-n_nodes // (P * ncores))      # 49
        self.npad = ncores * self.bpc * P
        self.nblk = ncores * self.bpc
        self.t1w = 384               # [h 256 | asrc 8 | adst 8 | junk]
        self.t2w = 128               # [h2 64 | asrc2 1 | adst2 1 | junk]
        self.glo = list(glo)         # per block position, len bpc
        self.ghi = list(ghi)
        self.g = [a + b for a, b in zip(self.glo, self.ghi)]
        self.sg = sum(self.g)
        self.gmax = max(self.g)


_GQ = [0]


def build_program(cfg):
    nc = bacc.Bacc(None, num_devices=cfg.ncores, num_swdge_queues=4,
                   dynamic_dma_scratch_size=16384)
    HC1, H1, CH1, C2 = cfg.hc1, cfg.h1, cfg.ch1, cfg.c2
    T1W, T2W, BPC = cfg.t1w, cfg.t2w, cfg.bpc
    NBLK, NPAD = cfg.nblk, cfg.npad
    G1 = 8
    G0 = -(-BPC // G1)              # phase-0 groups of 8 blocks

    # ---- I/O ----
    xt = nc.dram_tensor("xt", [cfg.c_in, NPAD], F16, kind="ExternalInput")
    w1aug = nc.dram_tensor("w1aug", [cfg.c_in, 272], F16, kind="ExternalInput")
    w2aug = nc.dram_tensor("w2aug", [HC1, 66], F16, kind="ExternalInput")
    b1b = nc.dram_tensor("b1b", [P, HC1], F16, kind="ExternalInput")
    b2b = nc.dram_tensor("b2b", [P, C2], F32, kind="ExternalInput")
    shifts = nc.dram_tensor("shifts", [P, 2], F32, kind="ExternalInput")
    ident = nc.dram_tensor("ident", [P, P], F16, kind="ExternalInput")
    padmask = nc.dram_tensor("padmask", [P, BPC], F16, kind="ExternalInput")
    eidx = nc.dram_tensor("eidx", [P, 8 * cfg.sg], I16, kind="ExternalInput")
    aidx = nc.dram_tensor("aidx", [P, 2 * BPC * 8], I16, kind="ExternalInput")
    sentadst = nc.dram_tensor("sentadst", [P, H1], F16, kind="ExternalInput")
    out = nc.dram_tensor("out", [BPC * P, C2], F32, kind="ExternalOutput")

    # ---- internal DRAM ----
    t1 = nc.dram_tensor("t1", [NPAD, T1W], F16)
    cc_in = nc.dram_tensor("cc_in", [BPC * P, T2W], F16)
    t2 = nc.dram_tensor("t2", [NPAD, T2W], F16, addr_space="Shared")

    groups = [list(range(cfg.ncores))]
    GM = cfg.gmax

    with tile.TileContext(nc) as tc:
        with (
            tc.tile_pool(name="const", bufs=1) as cpool,
            tc.tile_pool(name="p1", bufs=2) as p1pool,
            tc.tile_pool(name="gat", bufs=5) as gpool,
            tc.tile_pool(name="blk", bufs=3) as bpool,
            tc.tile_pool(name="fin", bufs=4) as opool,
            tc.tile_pool(name="ps", bufs=2, space="PSUM") as ps,
        ):
            # ---------------- constants ----------------
            ident_s = cpool.tile([P, P], F16)
            nc.sync.dma_start(out=ident_s[:], in_=ident[:])
            w1aug_s = cpool.tile([P, 272], F16)
            nc.sync.dma_start(out=w1aug_s[:], in_=w1aug[:])
            w2aug_s = []
            for j in range(HC1 // P):
                wg = cpool.tile([P, 66], F16, tag=f"w2aug{j}")
                nc.sync.dma_start(out=wg[:], in_=w2aug[j * P:(j + 1) * P, :])
                w2aug_s.append(wg)
            b1b_s = cpool.tile([P, HC1], F16)
            nc.sync.dma_start(out=b1b_s[:], in_=b1b[:])
            b2b_s = cpool.tile([P, C2], F32)
            nc.sync.dma_start(out=b2b_s[:], in_=b2b[:])
            shifts_s = cpool.tile([P, 2], F32)
            nc.sync.dma_start(out=shifts_s[:], in_=shifts[:])
            padmask_s = cpool.tile([P, BPC], F16)
            nc.sync.dma_start(out=padmask_s[:], in_=padmask[:])
            aidx_s = cpool.tile([P, 2 * BPC * 8], I16)
            nc.sync.dma_start(out=aidx_s[:], in_=aidx[:])
            sentadst_s = cpool.tile([P, H1], F16)
            nc.sync.dma_start(out=sentadst_s[:], in_=sentadst[:])
            eidx_s = cpool.tile([P, 8 * cfg.sg], I16)
            nc.sync.dma_start(out=eidx_s[:], in_=eidx[:])
            # persistent per-core tables
            A1 = cpool.tile([P, BPC * H1], F16)       # own-node a_dst
            A2 = cpool.tile([P, BPC], F16)            # own-node a_dst2

            # ------------- phase 1: full node table (replicated) ---------
            t1_writes = []
            for grp in range(NBLK // G1):
                B0 = grp * G1
                xg = p1pool.tile([P, G1 * P], F16, tag="xg")
                nc.sync.dma_start(out=xg[:], in_=xt[:, B0 * P:(B0 + G1) * P])
                rows = p1pool.tile([P, G1 * 272], F16, tag="rows")
                for j in range(G1):
                    ph1 = ps.tile([P, 272], F32, space="PSUM", tag="acc")
                    nc.tensor.matmul(out=ph1[:], lhsT=xg[:, j * P:(j + 1) * P],
                                     rhs=w1aug_s[:], start=True, stop=True)
                    dst = rows[:, j * 272:(j + 1) * 272]
                    if j % 2 == 0:
                        nc.scalar.copy(out=dst, in_=ph1[:])
                    else:
                        nc.vector.tensor_scalar_add(out=dst, in0=ph1[:],
                                                    scalar1=0.0)
                t1_writes.append(nc.sync.dma_start(
                    out=t1[B0 * P:(B0 + G1) * P, 0:272].rearrange(
                        "(j p) c -> p j c", j=G1),
                    in_=rows[:].rearrange("p (j c) -> p j c", j=G1)))

            j1tile = cpool.tile([1, 1], F32, tag="j1")
            j1 = nc.gpsimd.memset(j1tile[:], 0.0)
            for w in t1_writes:
                _dep(j1, w, "layer1 gathers wait for full node table")

            # ------------- phase 0b: own-node a_dst from t1 --------------
            # Per core all own rows sit in one table half; the other half's
            # call gathers only the sentinel row (idx data decides), and the
            # host-supplied sentinel a_dst is subtracted back out.
            MA = 8                               # block-cols per piece
            for k0 in range(0, BPC, MA):
                nk = min(MA, BPC - k0)
                bufs = []
                for half in range(2):
                    buf = p1pool.tile([P, MA * P], F16, tag=f"a1g{half}")
                    tslc = t1[SPLIT:NPAD, 256:384] if half \
                        else t1[0:SPLIT, 256:384]
                    q = _GQ[0] % 4
                    _GQ[0] += 1
                    g = nc.gpsimd.dma_gather(
                        out_ap=buf[:, 0:nk * P]
                            .rearrange("p (m w) -> p m w", m=nk),
                        in_ap=tslc,
                        idxs_ap=aidx_s[:, half * BPC * 8 + k0 * 8:
                                       half * BPC * 8 + (k0 + nk) * 8],
                        num_idxs=nk * P, num_idxs_reg=nk * P,
                        elem_size=P, elem_step=T1W, queue_num=q)
                    _dep(g, j1, "a_dst gather after t1")
                    bufs.append(buf)
                # adst cols are 8:16 of the gathered 128-col window
                asl = A1[:, k0 * H1:(k0 + nk) * H1]
                nc.vector.tensor_tensor(
                    out=asl.rearrange("p (b h) -> p b h", b=nk),
                    in0=bufs[0][:, 0:nk * P]
                        .rearrange("p (b w) -> p b w", b=nk)[:, :, 8:16],
                    in1=bufs[1][:, 0:nk * P]
                        .rearrange("p (b w) -> p b w", b=nk)[:, :, 8:16],
                    op=ALU.add)
                nc.vector.tensor_tensor(
                    out=asl.rearrange("p (b h) -> p b h", b=nk),
                    in0=asl.rearrange("p (b h) -> p b h", b=nk),
                    in1=sentadst_s[:].unsqueeze(1).to_broadcast([P, nk, H1]),
                    op=ALU.subtract)

            def tree_reduce(width, G, bufA, bufB):
                """Sum bufA[:, :G*width] over the G groups; ping-pongs
                between bufA and bufB. Returns the buffer whose cols
                [0:width] hold the sums."""
                src, dstb, n = bufA, bufB, G
                while n > 1:
                    h = n // 2
                    odd = n - 2 * h
                    nc.vector.tensor_tensor(
                        out=dstb[:, 0:h * width],
                        in0=src[:, 0:h * width],
                        in1=src[:, h * width:2 * h * width],
                        op=ALU.add)
                    if odd:
                        nc.vector.tensor_tensor(
                            out=dstb[:, 0:width],
                            in0=dstb[:, 0:width],
                            in1=src[:, 2 * h * width:(2 * h + 1) * width],
                            op=ALU.add)
                    src, dstb = dstb, src
                    n = h
                return src

            # ------------- phase 2: layer-1 per-dst aggregation ----------
            # Blocks are processed in column segments of <= GCAP so gather
            # tiles are small enough for a deep (bufs=4) pipeline; segment
            # partial sums accumulate into a per-block accumulator.
            W264 = HC1 + H1

            def segments(G):
                k = -(-G // GCAP)
                bounds = [2 * round(i * G / k / 2) for i in range(k + 1)]
                bounds[-1] = G
                return list(zip(bounds[:-1], bounds[1:]))

            def gather_seg(b, c0, c1, grow, tab, elem, dep, why):
                """Gather grid columns [c0, c1) of block b into grow."""
                glo = cfg.glo[b]
                ibase = 8 * sum(cfg.g[:b])
                nseg = c1 - c0
                ranges = []
                if c0 < glo:
                    ranges.append((c0, min(c1, glo), 0))
                if c1 > glo:
                    ranges.append((max(c0, glo), c1, 1))
                for (r0_, r1_, hi) in ranges:
                    tslc = tab[SPLIT:NPAD, 0:elem] if hi \
                        else tab[0:SPLIT, 0:elem]
                    for k0 in range(r0_, r1_, MAXCOLS):
                        nk = min(MAXCOLS, r1_ - k0)
                        q = _GQ[0] % 4
                        _GQ[0] += 1
                        g = nc.gpsimd.dma_gather(
                            out_ap=grow[:, (k0 - c0) * elem:
                                        (k0 - c0 + nk) * elem]
                                .rearrange("p (m w) -> p m w", m=nk),
                            in_ap=tslc,
                            idxs_ap=eidx_s[:, ibase + k0 * 8:
                                           ibase + (k0 + nk) * 8],
                            num_idxs=nk * 128, num_idxs_reg=nk * 128,
                            elem_size=elem, queue_num=q)
                        _dep(g, dep, why)

            cc_writes = []
            for b in range(BPC):
                G = cfg.g[b]
                r0 = b * P
                segs = segments(G)
                acc = opool.tile([P, W264], F16, tag="acc1")
                for si, (c0, c1) in enumerate(segs):
                    ncol = c1 - c0
                    grow = gpool.tile([P, GCAP * T1W], F16, tag="grow")
                    gather_seg(b, c0, c1, grow, t1, T1W, j1, "l1 gather")
                    grv = grow[:, :ncol * T1W].rearrange(
                        "p (g c) -> p g c", g=ncol)

                    av = bpool.tile([P, GCAP * H1], F16, tag="av")
                    nc.vector.tensor_tensor(
                        out=av[:, :ncol * H1].rearrange(
                            "p (g h) -> p g h", g=ncol),
                        in0=grv[:, :, 256:264],
                        in1=A1[:, b * H1:(b + 1) * H1].unsqueeze(1)
                            .to_broadcast([P, ncol, H1]),
                        op=ALU.add)
                    lk = bpool.tile([P, GCAP * H1], F16, tag="lk")
                    nc.vector.scalar_tensor_tensor(
                        out=lk[:, :ncol * H1], in0=av[:, :ncol * H1],
                        scalar=NEG_SLOPE, in1=av[:, :ncol * H1],
                        op0=ALU.mult, op1=ALU.max)

                    wm = bpool.tile([P, GCAP * W264], F16, tag="wm")
                    wmv = wm[:, :ncol * W264].rearrange(
                        "p (g c) -> p g c", g=ncol)
                    # exp, broadcast 8 heads -> 256 lanes, on the scalar
                    # engine, directly into wm's message columns
                    nc.scalar.activation(
                        out=wmv[:, :, 0:HC1].rearrange(
                            "p g (h c) -> p g h c", h=H1),
                        in_=lk[:, :ncol * H1].rearrange(
                            "p (g h) -> p g h", g=ncol)
                            .unsqueeze(3).to_broadcast([P, ncol, H1, CH1]),
                        func=AF.Exp, bias=shifts_s[:, 0:1])
                    # denominator columns: one e per head (stride-32 picks)
                    nc.vector.tensor_copy(
                        out=wmv[:, :, HC1:W264],
                        in_=wmv[:, :, 0:HC1].rearrange(
                            "p g (h c) -> p g h c", h=H1)[:, :, :, 0])
                    nc.vector.tensor_tensor(
                        out=wmv[:, :, 0:HC1],
                        in0=grv[:, :, 0:HC1],
                        in1=wmv[:, :, 0:HC1],
                        op=ALU.mult)

                    redt = bpool.tile([P, (GCAP // 2 + 1) * W264], F16,
                                      tag="redt")
                    red = tree_reduce(W264, ncol, wm, redt)
                    if si == 0:
                        nc.vector.tensor_copy(out=acc[:],
                                              in_=red[:, 0:W264])
                    else:
                        nc.vector.tensor_tensor(
                            out=acc[:], in0=acc[:], in1=red[:, 0:W264],
                            op=ALU.add)

                msum = acc[:, 0:HC1]
                dsum = acc[:, HC1:W264]
                denf = opool.tile([P, H1], F32, tag="denf")
                nc.vector.tensor_scalar_add(out=denf[:], in0=dsum,
                                            scalar1=EPS)
                rec = opool.tile([P, H1], F32, tag="rec")
                nc.vector.reciprocal(out=rec[:], in_=denf[:])
                recb = opool.tile([P, HC1], F16, tag="recb")
                nc.vector.tensor_scalar(
                    out=recb[:].rearrange("p (h c) -> p h c", h=H1),
                    in0=rec[:].unsqueeze(2).to_broadcast([P, H1, CH1]),
                    scalar1=60000.0, scalar2=None, op0=ALU.min)
                o1b = opool.tile([P, HC1], F16, tag="o1b")
                nc.vector.tensor_tensor(out=o1b[:], in0=msum, in1=recb[:],
                                        op=ALU.mult)
                nc.vector.tensor_tensor(out=o1b[:], in0=o1b[:], in1=b1b_s[:],
                                        op=ALU.add)
                xn = opool.tile([P, HC1], F16, tag="recb")
                nc.vector.tensor_scalar_min(out=xn[:], in0=o1b[:], scalar1=0.0)
                en = opool.tile([P, HC1], F16, tag="recb")
                nc.scalar.activation(out=en[:], in_=xn[:], func=AF.Exp)
                helu = opool.tile([P, HC1], F16, tag="o1b")
                nc.vector.scalar_tensor_tensor(
                    out=helu[:], in0=o1b[:], scalar=0.0, in1=en[:],
                    op0=ALU.max, op1=ALU.add)

                ph2 = ps.tile([P, 66], F32, space="PSUM", tag="ph2")
                for j in range(HC1 // P):
                    pT = ps.tile([P, P], F16, space="PSUM", tag="pT")
                    nc.tensor.transpose(out=pT[:],
                                        in_=helu[:, j * P:(j + 1) * P],
                                        identity=ident_s[:])
                    hT = opool.tile([P, P], F16, tag="hT")
                    nc.scalar.copy(out=hT[:], in_=pT[:])
                    nc.tensor.matmul(out=ph2[:], lhsT=hT[:], rhs=w2aug_s[j][:],
                                     start=(j == 0), stop=(j == HC1 // P - 1))
                h2row = opool.tile([P, 66], F16, tag="h2row")
                nc.scalar.copy(out=h2row[:, 0:66], in_=ph2[:])
                # force sentinel/pad-node layer-2 a_src very negative
                nc.vector.tensor_tensor(
                    out=h2row[:, 64:65], in0=h2row[:, 64:65],
                    in1=padmask_s[:, b:b + 1], op=ALU.add)
                nc.scalar.copy(out=A2[:, b:b + 1], in_=h2row[:, 65:66])
                cc_writes.append(nc.sync.dma_start(
                    out=cc_in[r0:r0 + P, 0:66], in_=h2row[:]))

            # ------------- phase 3: share layer-2 node table -------------
            cc = nc.gpsimd.collective_compute(
                "AllGather", ALU.bypass, replica_groups=groups,
                ins=[cc_in[:]], outs=[t2[:]])
            for w in cc_writes:
                _dep(cc, w, "allgather after cc writes")
            j2tile = cpool.tile([1, 1], F32, tag="j2")
            j2 = nc.gpsimd.memset(j2tile[:], 0.0)
            _dep(j2, cc, "layer2 gathers after allgather")

            # ------------- phase 4: layer-2 per-dst aggregation ----------
            for b in range(BPC):
                G = cfg.g[b]
                r0 = b * P
                segs = segments(G)
                acc2 = opool.tile([P, C2], F16, tag="acc2")
                acc2d = opool.tile([P, 2], F32, tag="acc2d")
                for si, (c0, c1) in enumerate(segs):
                    ncol = c1 - c0
                    grow2 = gpool.tile([P, GCAP * T1W], F16, tag="grow")
                    gather_seg(b, c0, c1, grow2, t2, T2W, j2, "l2 gather")
                    grv2 = grow2[:, :ncol * T2W].rearrange(
                        "p (g c) -> p g c", g=ncol)

                    av2 = bpool.tile([P, GCAP], F16, tag="av2")
                    nc.vector.scalar_tensor_tensor(
                        out=av2[:, :ncol],
                        in0=grv2[:, :, 64:65].rearrange("p g o -> p (g o)"),
                        scalar=shifts_s[:, 1:2],
                        in1=A2[:, b:b + 1].to_broadcast([P, ncol]),
                        op0=ALU.add, op1=ALU.add)
                    lk2 = bpool.tile([P, GCAP], F16, tag="lk2")
                    nc.vector.scalar_tensor_tensor(
                        out=lk2[:, :ncol], in0=av2[:, :ncol], scalar=NEG_SLOPE,
                        in1=av2[:, :ncol], op0=ALU.mult, op1=ALU.max)
                    e2 = bpool.tile([P, GCAP], F16, tag="e2")
                    nc.scalar.activation(out=e2[:, :ncol], in_=lk2[:, :ncol],
                                         func=AF.Exp, bias=shifts_s[:, 0:1])

                    wm2 = bpool.tile([P, GCAP * C2], F16, tag="wm")
                    wm2v = wm2[:, :ncol * C2].rearrange(
                        "p (g c) -> p g c", g=ncol)
                    nc.vector.tensor_tensor(
                        out=wm2v[:],
                        in0=grv2[:, :, 0:C2],
                        in1=e2[:, :ncol].unsqueeze(2)
                            .to_broadcast([P, ncol, C2]),
                        op=ALU.mult)

                    redt2 = bpool.tile([P, (GCAP // 2 + 1) * C2], F16,
                                       tag="redt")
                    red2 = tree_reduce(C2, ncol, wm2, redt2)
                    dpart = opool.tile([P, 2], F32, tag="dpart")
                    nc.vector.tensor_reduce(
                        out=dpart[:, 0:1], in_=e2[:, :ncol],
                        axis=mybir.AxisListType.X, op=ALU.add)
                    if si == 0:
                        nc.vector.tensor_copy(out=acc2[:], in_=red2[:, 0:C2])
                        nc.vector.tensor_copy(out=acc2d[:, 0:1],
                                              in_=dpart[:, 0:1])
                    else:
                        nc.vector.tensor_tensor(
                            out=acc2[:], in0=acc2[:], in1=red2[:, 0:C2],
                            op=ALU.add)
                        nc.vector.tensor_tensor(
                            out=acc2d[:, 0:1], in0=acc2d[:, 0:1],
                            in1=dpart[:, 0:1], op=ALU.add)

                den2f = opool.tile([P, 1], F32, tag="den2f")
                nc.vector.tensor_scalar_add(out=den2f[:],
                                            in0=acc2d[:, 0:1],
                                            scalar1=EPS)
                rec2 = opool.tile([P, 1], F32, tag="rec2")
                nc.vector.reciprocal(out=rec2[:], in_=den2f[:])
                o2 = opool.tile([P, C2], F32, tag="o2")
                nc.vector.scalar_tensor_tensor(
                    out=o2[:], in0=acc2[:], scalar=rec2[:, 0:1],
                    in1=b2b_s[:], op0=ALU.mult, op1=ALU.add)
                nc.sync.dma_start(out=out[r0:r0 + P, :], in_=o2[:])

    nc.compile()
    return nc


def _wrap16(idx):
    """Pack int16 idx array (len multiple of 128) into wrapped-16 layout
    [128, n//16]: element k at (k%16, k//16), replicated to rows 16..127."""
    n = len(idx)
    a = np.asarray(idx, np.int16).reshape(n // 16, 16).T  # [16, n//16]
    return np.tile(a, (8, 1))


def _deal_half(order, ranked_nodes, id0, half_cores, bpc):
    """Deal deg-sorted nodes of one class into its 4 cores' id range:
    consecutive 128-node blocks go to (core stripe, position) pairs."""
    nh = len(ranked_nodes)
    r = np.arange(nh)
    blk = r >> 7
    ids = (id0 + ((blk % half_cores) * bpc + blk // half_cores) * P
           + (r & 127))
    order[ids] = ranked_nodes


def compute_layout(n, edge_index):
    """Node permutation + per-position (Glo, Ghi) column counts."""
    bpc = -(-n // (P * NCORES))
    npad = NCORES * bpc * P
    nblk = NCORES * bpc
    half = npad // 2                 # == SPLIT for the 8-core layout
    assert half == SPLIT

    src = np.asarray(edge_index[0]).astype(np.int64)
    dst = np.asarray(edge_index[1]).astype(np.int64)
    deg = np.bincount(dst, minlength=n) + 1      # + self loop

    # fix each node's lo/hi CLASS up front (alternating deg-rank blocks
    # so both halves get identical degree profiles); classes never move,
    # so per-node lo-counts are exact, not a fixed-point guess.
    rank_of = np.argsort(-deg, kind="stable")    # rank -> orig node
    rank_inv = np.empty(n, np.int64)
    rank_inv[rank_of] = np.arange(n)
    is_lo = ((rank_inv >> 7) % NCORES) < (NCORES // 2)   # per orig node
    # keep >=1 pad id in each half for its sentinel row
    for flip_from, flag in ((is_lo, True), (~is_lo, False)):
        excess = int(flip_from.sum()) - (half - 1)
        if excess > 0:
            cand = np.nonzero(flip_from)[0]
            worst = cand[np.argsort(rank_inv[cand])[-excess:]]
            is_lo[worst] = not flag

    lo_cnt = np.bincount(dst[is_lo[src]], minlength=n)
    lo_cnt += is_lo                               # self loop
    hi_cnt = deg - lo_cnt
    key = lo_cnt.astype(np.int64) * (4 * npad) + hi_cnt
    krank = np.argsort(key, kind="stable")

    order = np.full(npad, -1, np.int64)           # new id -> orig node
    lo_nodes = krank[is_lo[krank]]
    hi_nodes = krank[~is_lo[krank]]
    assert len(lo_nodes) <= half and len(hi_nodes) <= half
    _deal_half(order, lo_nodes, 0, NCORES // 2, bpc)
    _deal_half(order, hi_nodes, half, NCORES // 2, bpc)

    # force sentinel ids (last row of each table half) to be pads
    for sent, lim0, lim1 in ((SPLIT - 1, 0, half), (npad - 1, half, npad)):
        if order[sent] >= 0:
            padq = np.nonzero(order[lim0:lim1] < 0)[0]
            assert len(padq), "no pad id available in half"
            q = lim0 + padq[-1]
            order[q] = order[sent]
            order[sent] = -1
    new_id = np.full(n, -1, np.int64)
    real = np.nonzero(order >= 0)[0]
    new_id[order[real]] = real
    # classes preserved by construction
    assert (new_id[lo_nodes] < SPLIT).all()
    assert (new_id[hi_nodes] >= SPLIT).all()

    # exact per-block lo/hi maxima under the final assignment
    src_n = new_id[src]
    dst_n = new_id[dst]
    lo_edge = src_n < SPLIT
    lo_c = np.bincount(dst_n[lo_edge], minlength=npad)
    hi_c = np.bincount(dst_n[~lo_edge], minlength=npad)
    sl = np.nonzero(order >= 0)[0]               # self loops (new ids)
    np.add.at(lo_c, sl[sl < SPLIT], 1)
    np.add.at(hi_c, sl[sl >= SPLIT], 1)

    lo_blk = lo_c.reshape(nblk, P).max(axis=1)
    hi_blk = hi_c.reshape(nblk, P).max(axis=1)
    glo = lo_blk.reshape(NCORES, bpc).max(axis=0)
    ghi = hi_blk.reshape(NCORES, bpc).max(axis=0)
    glo = glo.astype(np.int64)
    ghi = ghi.astype(np.int64)
    for i in range(bpc):
        if (glo[i] + ghi[i]) % 2:
            ghi[i] += 1
        if glo[i] + ghi[i] == 0:
            ghi[i] = 2
    return order, new_id, [int(v) for v in glo], [int(v) for v in ghi]


def host_prep(cfg, x, W1, att_src1, att_dst1, bias1, W2, att_src2,
              att_dst2, bias2, edge_index):
    n = cfg.n
    NPAD, BPC = cfg.npad, cfg.bpc
    H1, CH1, HC1 = cfg.h1, cfg.ch1, cfg.hc1
    order, new_id = cfg.order, cfg.new_id

    src = np.asarray(edge_index[0]).astype(np.int64)
    dst = np.asarray(edge_index[1]).astype(np.int64)
    loop = np.arange(n, dtype=np.int64)
    src = np.concatenate([src, loop])
    dst = np.concatenate([dst, loop])
    src_n = new_id[src]
    dst_n = new_id[dst]

    # group edges by new dst, lo-src first within each node
    lo_flag = src_n < SPLIT
    eorder = np.argsort(dst_n * 2 + (~lo_flag), kind="stable")
    src_s = src_n[eorder]
    dst_s = dst_n[eorder]
    lo_s = lo_flag[eorder]

    counts = np.bincount(dst_s, minlength=NPAD)
    lo_cnt = np.bincount(dst_s[lo_s], minlength=NPAD)
    starts = np.zeros(NPAD + 1, np.int64)
    np.cumsum(counts, out=starts[1:])
    pos_in_node = np.arange(len(src_s)) - starts[dst_s]

    SENT_LO = SPLIT - 1
    SENT_HI_REL = (NPAD - 1) - SPLIT
    blk_of = dst_s >> 7
    part_of = dst_s & 127

    SG8 = 8 * cfg.sg
    eidx_cores = [np.empty((P, SG8), np.int16) for _ in range(NCORES)]
    ibase = 0
    for bpos in range(BPC):
        glo, ghi = cfg.glo[bpos], cfg.ghi[bpos]
        G = glo + ghi
        for c in range(NCORES):
            blk = c * BPC + bpos
            grid = np.empty((P, G), np.int16)
            grid[:, :glo] = SENT_LO
            grid[:, glo:] = SENT_HI_REL
            m = blk_of == blk
            pp = part_of[m]
            sv = src_s[m]
            lv = lo_s[m]
            pn = pos_in_node[m]
            ln = lo_cnt[blk * P + pp]
            col = np.where(lv, pn, glo + (pn - ln))
            grid[pp, col] = np.where(lv, sv, sv - SPLIT).astype(np.int16)
            flat = grid.T.reshape(-1)            # k = g*128 + p
            eidx_cores[c][:, ibase:ibase + G * 8] = _wrap16(flat)
        ibase += G * 8

    # ---- parameter prep ----
    x = np.asarray(x, np.float32)
    W1 = np.asarray(W1, np.float32)
    W2 = np.asarray(W2, np.float32)
    as1 = np.asarray(att_src1, np.float32)
    ad1 = np.asarray(att_dst1, np.float32)
    as2 = np.asarray(att_src2, np.float32).reshape(-1)
    ad2 = np.asarray(att_dst2, np.float32).reshape(-1)

    A1s = np.zeros((HC1, H1), dtype=np.float32)
    A1d = np.zeros((HC1, H1), dtype=np.float32)
    hh = np.repeat(np.arange(H1), CH1)
    A1s[np.arange(HC1), hh] = as1.reshape(-1)
    A1d[np.arange(HC1), hh] = ad1.reshape(-1)
    Bs = W1 @ A1s                                   # [c_in, H1]
    # sentinel x: a_src(x_sent) == -100 per head, minimal norm
    x_sent = Bs @ np.linalg.solve(Bs.T @ Bs, -100.0 * np.ones(H1))

    xp = np.empty((NPAD, cfg.c_in), np.float32)
    real = order >= 0
    xp[real] = x[order[real]]
    xp[~real] = x_sent
    xt = np.ascontiguousarray(xp.T).astype(np.float16)

    w1aug = np.concatenate([W1, Bs, W1 @ A1d], axis=1).astype(np.float16)
    w2aug = np.concatenate([W2, (W2 @ as2)[:, None], (W2 @ ad2)[:, None]],
                           axis=1).astype(np.float16)

    colsum = W2.sum(axis=0)
    c0 = float(colsum @ (as2 + ad2))
    shifts = np.zeros((P, 2), dtype=np.float32)
    shifts[:, 0] = -2.0
    shifts[:, 1] = -c0

    b1b = np.tile(np.asarray(bias1, np.float32).reshape(1, -1),
                  (P, 1)).astype(np.float16)
    b2b = np.tile((np.asarray(bias2, np.float32).reshape(-1) - colsum
                   ).reshape(1, -1), (P, 1)).astype(np.float32)
    ident = np.eye(P, dtype=np.float16)

    # sentinel a_dst as the device computes it (fp16 inputs, fp32 matmul)
    sentadst_v = (x_sent.astype(np.float16).astype(np.float32)
                  @ w1aug[:, 264:272].astype(np.float32)).astype(np.float16)
    sentadst = np.tile(sentadst_v.reshape(1, -1), (P, 1))

    SENT_HI = (NPAD - 1) - SPLIT
    in_maps = []
    for c in range(NCORES):
        base = c * BPC * P
        pm = np.zeros((P, BPC), np.float16)
        ids = (base + np.arange(BPC)[None, :] * P +
               np.arange(P)[:, None])
        pm[:, :] = np.where(order[ids] < 0, -1000.0, 0.0)
        # own-row a_dst gather indices (lo call | hi call)
        own = base + np.arange(BPC * P, dtype=np.int64)
        if base + BPC * P <= SPLIT:
            alo, ahi = own, np.full(BPC * P, SENT_HI, np.int64)
        else:
            alo = np.full(BPC * P, SPLIT - 1, np.int64)
            ahi = own - SPLIT
        aidxv = np.concatenate(
            [_wrap16(alo.astype(np.int16)), _wrap16(ahi.astype(np.int16))],
            axis=1)
        in_maps.append({
            "xt": xt, "w1aug": w1aug, "w2aug": w2aug, "b1b": b1b,
            "b2b": b2b, "shifts": shifts, "ident": ident,
            "padmask": pm, "eidx": eidx_cores[c], "aidx": aidxv,
            "sentadst": sentadst})
    return in_maps


_prog_cache = {}
_last_results = None


def kernel(x, edge_index, edge_weight, W1, att_src1, att_dst1, bias1,
           W2, att_src2, att_dst2, bias2):
    global _last_results
    n = x.shape[0]
    # edge_weight is unused (GATConv with edge_dim=None ignores it)
    order, new_id, glo, ghi = compute_layout(n, edge_index)
    cfg = Cfg(n, glo, ghi, c_in=x.shape[1])
    cfg.order, cfg.new_id = order, new_id
    key = (cfg.n, cfg.c_in, tuple(glo), tuple(ghi))
    if key not in _prog_cache:
        _prog_cache[key] = build_program(cfg)
    nc = _prog_cache[key]

    in_maps = host_prep(cfg, x, W1, att_src1, att_dst1, bias1, W2,
                        att_src2, att_dst2, bias2, edge_index)
    res = run_bass_kernel_spmd(nc, in_maps, list(range(cfg.ncores)))
    _last_results = res
    full = np.concatenate([res.results[c]["out"]
                           for c in range(cfg.ncores)], axis=0)
    out = np.zeros((n, cfg.c2), np.float32)
    real = order >= 0
    out[order[real]] = full[real]
    return np.ascontiguousarray(out)
